# revision 98
# baseline (speedup 1.0000x reference)
"""AttentionBlock (GroupNorm + single-head 4096x4096 attention + out-proj) on 8
Trainium2 NeuronCores, data-parallel over batch (B=8 -> 1 image per core).

Numerics: scores q^T k must be fp32-grade (softmax over scores with sigma~2000
behaves like an argmax; bf16/fp32r-level score error flips argmaxes and fails).
Strategy: fp16 hi/lo split => q^T k = qh^T kh + ql^T kh + qh^T kl (3 matmuls at
1 cyc/row beats native fp32's 4 cyc/row). q,k projections likewise fp16-split.
The post-softmax path (e, attn, v, AV matmul) is bf16-safe. exp runs on ScalarE
with per-row bias=-max and accum_out giving the denominator for free; the
attention matrix is transposed for the AV matmul with DMA-xbar transposes.
"""
import sys
if "/opt/trn_rl_repo" not in sys.path:
    sys.path.insert(0, "/opt/trn_rl_repo")

from contextlib import ExitStack

import numpy as np

import concourse.bacc as bacc
import concourse.tile as tile
from concourse import mybir
from concourse.bass_utils import run_bass_kernel_spmd

B, C, H, W = 8, 256, 64, 64
HW = H * W            # 4096
G = 32                # groups
GS = C // G           # 8 channels / group
EPS = 1e-5
NT = HW // 128        # 32 i-tiles
NB = NT // 2          # 16 AV blocks of 256 columns

f32 = mybir.dt.float32
f8e4 = mybir.dt.float8e4
f16 = mybir.dt.float16
bf16 = mybir.dt.bfloat16

_PROGRAMS = {}


def _build_program(qk_bias, v_bias, fp8_cross=False):
    nc = bacc.Bacc("TRN2", target_bir_lowering=False, debug=False)

    def din(name, shape, dt=f32):
        return nc.dram_tensor(name, shape, dt, kind="ExternalInput").ap()

    d_x = din("x", (C, HW))
    # weights pre-transposed to [cin, cout]; q side pre-scaled by -0.5*C
    if qk_bias:
        d_wq_h, d_wq_l = din("wq_h", (C, C), f16), din("wq_l", (C, C), f16)
        d_wk_h, d_wk_l = din("wk_h", (C, C), f16), din("wk_l", (C, C), f16)
    else:
        # zero q/k bias: scores fold to u^T hn with u = M2^T hn, M2 = Wq'^T Wk
        d_m2_h, d_m2_l = din("m2_h", (C, C), f16), din("m2_l", (C, C), f16)
    d_wv_h = din("wv_h", (C, C), f16)
    d_wo_h = din("wo_h", (C, C), f16)
    d_gamma, d_beta = din("gamma", (C,)), din("beta", (C,))
    d_bq = din("bq", (1, 2, C), f16)   # pre-scaled by -0.5*C, [hi, lo]
    d_bk = din("bk", (1, 2, C), f16)   # [hi, lo]
    d_bv = din("bv", (1, C), f16)
    d_bo = din("bo", (C,))
    d_gmat = din("gmat", (128, 128))  # block-diag 1/GS group-averaging matrix
    d_out = nc.dram_tensor("out", (C, HW), f32, kind="ExternalOutput").ap()

    with tile.TileContext(nc) as tc, ExitStack() as ctx:
        main = ctx.enter_context(tc.tile_pool(name="main", bufs=1))
        psA = ctx.enter_context(tc.tile_pool(name="psA", bufs=3, space="PSUM"))
        psB = ctx.enter_context(tc.tile_pool(name="psB", bufs=2, space="PSUM"))

        # ---------------- persistent tiles ----------------
        wv_h = main.tile([128, 2, C], f16, name="wv_h")
        wo_h = main.tile([128, 2, C], f16, name="wo_h")
        # wo_h first and via SWDGE: it feeds the very first PE matmuls and
        # must not queue behind the x-chunk loads on the HWDGE queues
        nc.gpsimd.dma_start(wo_h, d_wo_h.rearrange("(kc kl) m -> kl kc m", kl=128))
        if qk_bias:
            wq_h = main.tile([128, 2, C], f16, name="wq_h")
            wq_l = main.tile([128, 2, C], f16, name="wq_l")
            wk_h = main.tile([128, 2, C], f16, name="wk_h")
            wk_l = main.tile([128, 2, C], f16, name="wk_l")
            wloads = [(wk_h, d_wk_h), (wk_l, d_wk_l), (wq_h, d_wq_h),
                      (wq_l, d_wq_l), (wv_h, d_wv_h)]
        else:
            m2_h = main.tile([128, 2, C], f16, name="m2_h")
            m2_l = main.tile([128, 2, C], f16, name="m2_l")
            wloads = [(m2_h, d_m2_h), (m2_l, d_m2_l), (wv_h, d_wv_h)]
        for t, d in wloads:
            nc.sync.dma_start(t, d.rearrange("(kc kl) m -> kl kc m", kl=128))

        # score-matmul operand pairs: (q,k) hi/lo, or (u, hn) hi/lo
        qh = main.tile([128, 2, HW], f16, name="qh")
        ql = None if fp8_cross else main.tile([128, 2, HW], f16, name="ql")
        ul8 = hh8 = None
        if fp8_cross:
            ul8 = main.tile([128, 2, HW], f8e4, name="ul8")
            hh8 = main.tile([128, 2, HW], f8e4, name="hh8")
        if qk_bias:
            kh = main.tile([128, 2, HW], f16, name="kh")
            kl = main.tile([128, 2, HW], f16, name="kl")
            hn_pool = None
        else:
            hn_pool = main   # hn hi/lo persists as the scores rhs
        vT = main.tile([128, NT, C], bf16, name="vT")

        gmat = main.tile([128, 128], f32, name="gmat")
        nc.sync.dma_start(gmat, d_gmat)
        gamma_sb = main.tile([128, 2], f32, name="gamma_sb")
        beta_sb = main.tile([128, 2], f32, name="beta_sb")
        bo_sb = main.tile([128, 2], f32, name="bo_sb")
        nc.sync.dma_start(gamma_sb, d_gamma.rearrange("(t p) -> p t", p=128))
        nc.sync.dma_start(beta_sb, d_beta.rearrange("(t p) -> p t", p=128))
        nc.sync.dma_start(bo_sb, d_bo.rearrange("(t p) -> p t", p=128))
        bq_row = main.tile([1, 2, C], f16, name="bq_row")
        bk_row = main.tile([1, 2, C], f16, name="bk_row")
        bv_row = main.tile([1, C], f16, name="bv_row")
        nc.sync.dma_start(bq_row, d_bq)
        nc.sync.dma_start(bk_row, d_bk)
        nc.sync.dma_start(bv_row, d_bv)
        ones_row = main.tile([1, 512], f16, name="ones_row")
        nc.vector.memset(ones_row, 1.0)
        eps_t = main.tile([128, 2], f32, name="eps_t")
        nc.vector.memset(eps_t, EPS)

        proj_sb = main.tile([128, 2, HW], f16, name="proj_sb")

        with tc.tile_pool(name="pre", bufs=1) as pre:
            # -------------- load x, GroupNorm stats --------------
            # chunked loads/casts so downstream work starts on chunk 0 early
            x_sb = pre.tile([128, 2, HW], f32, name="x_sb")
            d_xv = d_x.rearrange("(t p) n -> p t n", p=128)
            for c8 in range(8):
                for t in range(2):
                    n0 = c8 * 512
                    # first pair via the ACT HWDGE queues: they gate the first
                    # xh cast and all PE work, so skip the SP queue pile-up
                    eng = nc.scalar if c8 == 0 else nc.sync
                    eng.dma_start(x_sb[:, t, n0:n0 + 512],
                                  d_xv[:, t, n0:n0 + 512])

            xh = pre.tile([128, 2, HW], f16, name="xh")
            for c8 in range(8):
                for t in range(2):
                    n0 = c8 * 512
                    nc.scalar.activation(xh[:, t, n0:n0 + 512],
                                         x_sb[:, t, n0:n0 + 512],
                                         mybir.ActivationFunctionType.Copy)

            stats = pre.tile([128, 2, 8, 6], f32, name="stats")
            for t in range(2):
                xv = x_sb[:, t, :].rearrange("p (s n) -> p s n", n=512)
                for s in range(8):
                    nc.vector.bn_stats(stats[:, t, s, :], xv[:, s, :])
            # per-tile stats pipeline: tile 0's scale/bias is ready before
            # tile 1's stats finish, unblocking kc=0 projection matmuls early
            mv = pre.tile([128, 2, 2], f32, name="mv")
            a_sc = pre.tile([128, 2], f32, name="a_sc")
            b_sc = pre.tile([128, 2], f32, name="b_sc")
            stat2 = pre.tile([128, 2, 2], f32, name="stat2")
            gstat = pre.tile([128, 2, 2], f32, name="gstat")
            gvar = pre.tile([128, 2], f32, name="gvar")
            seps = pre.tile([128, 2], f32, name="seps")
            rstd = pre.tile([128, 2], f32, name="rstd")
            tmp = pre.tile([128, 2], f32, name="tmp")
            for t in range(2):
                nc.vector.bn_aggr(mv[:, t, :], stats[:, t, :, :])
                # stat2[:, t] = [mean_t, m2_t]
                nc.vector.tensor_tensor(out=stat2[:, t, 1:2], in0=mv[:, t, 0:1],
                                        in1=mv[:, t, 0:1], op=mybir.AluOpType.mult)
                nc.vector.tensor_tensor(out=stat2[:, t, 1:2], in0=stat2[:, t, 1:2],
                                        in1=mv[:, t, 1:2], op=mybir.AluOpType.add)
                nc.gpsimd.tensor_copy(stat2[:, t, 0:1], mv[:, t, 0:1])
                ps_g = psB.tile([128, 2], f32, name="ps_g", tag="psB")
                nc.tensor.matmul(ps_g, gmat, stat2[:, t, :], start=True, stop=True)
                nc.vector.tensor_copy(gstat[:, t, :], ps_g)
                gm = gstat[:, t, 0:1]
                nc.vector.tensor_tensor(out=gvar[:, t:t + 1], in0=gm, in1=gm,
                                        op=mybir.AluOpType.mult)
                nc.vector.tensor_tensor(out=gvar[:, t:t + 1], in0=gstat[:, t, 1:2],
                                        in1=gvar[:, t:t + 1],
                                        op=mybir.AluOpType.subtract)
                nc.vector.tensor_tensor(out=seps[:, t:t + 1], in0=gvar[:, t:t + 1],
                                        in1=eps_t[:, t:t + 1], op=mybir.AluOpType.add)
                nc.scalar.activation(rstd[:, t:t + 1], seps[:, t:t + 1],
                                     mybir.ActivationFunctionType.Sqrt)
                nc.vector.reciprocal(rstd[:, t:t + 1], rstd[:, t:t + 1])
                for _ in range(2):
                    nc.vector.tensor_tensor(out=tmp[:, t:t + 1], in0=rstd[:, t:t + 1],
                                            in1=rstd[:, t:t + 1], op=mybir.AluOpType.mult)
                    nc.vector.tensor_tensor(out=tmp[:, t:t + 1], in0=tmp[:, t:t + 1],
                                            in1=seps[:, t:t + 1], op=mybir.AluOpType.mult)
                    nc.vector.tensor_scalar(tmp[:, t:t + 1], tmp[:, t:t + 1], -0.5, 1.5,
                                            op0=mybir.AluOpType.mult,
                                            op1=mybir.AluOpType.add)
                    nc.vector.tensor_tensor(out=rstd[:, t:t + 1], in0=rstd[:, t:t + 1],
                                            in1=tmp[:, t:t + 1], op=mybir.AluOpType.mult)
                nc.vector.tensor_tensor(out=a_sc[:, t:t + 1], in0=rstd[:, t:t + 1],
                                        in1=gamma_sb[:, t:t + 1], op=mybir.AluOpType.mult)
                nc.vector.tensor_tensor(out=b_sc[:, t:t + 1], in0=gstat[:, t, 0:1],
                                        in1=a_sc[:, t:t + 1], op=mybir.AluOpType.mult)
                nc.vector.tensor_tensor(out=b_sc[:, t:t + 1], in0=beta_sb[:, t:t + 1],
                                        in1=b_sc[:, t:t + 1], op=mybir.AluOpType.subtract)

            # -------------- out-projection Wo@x + bo -> DRAM scratch ----------
            # (runs on PE while DVE computes hn; uses xh which is pre-norm x)
            for nch in range(8):
                n0 = nch * 512
                for mc in range(2):
                    ps_p = psA.tile([128, 1024], f32, name="ps_p", tag="psA")
                    for kc in range(2):
                        nc.tensor.matmul(ps_p[:, 0:512],
                                         wo_h[:, kc, mc * 128:(mc + 1) * 128],
                                         xh[:, kc, n0:n0 + 512],
                                         start=(kc == 0), stop=(kc == 1))
                    nc.vector.tensor_scalar(proj_sb[:, mc, n0:n0 + 512],
                                            ps_p[:, 0:512],
                                            bo_sb[:, mc:mc + 1], None,
                                            op0=mybir.AluOpType.add)

            # -------------- hn (in-place) and fp16 hi/lo split --------------
            # chunked so the q/k projections can start on early chunks
            hp = hn_pool if hn_pool is not None else pre
            hnh = hp.tile([128, 2, HW], f16, name="hnh")
            hnl = hp.tile([128, 2, HW], f16, name="hnl")
            for t in range(2):
                for c4 in range(4):
                    n0 = c4 * 1024
                    sl = (slice(None), t, slice(n0, n0 + 1024))
                    nc.vector.tensor_scalar(x_sb[sl], x_sb[sl],
                                            a_sc[:, t:t + 1], b_sc[:, t:t + 1],
                                            op0=mybir.AluOpType.mult,
                                            op1=mybir.AluOpType.add)
                    nc.vector.tensor_copy(hnh[sl], x_sb[sl])
                    nc.vector.tensor_tensor(out=hnl[sl], in0=x_sb[sl], in1=hnh[sl],
                                            op=mybir.AluOpType.subtract)

            # -------------- q,k projections (fp16-split, exact-grade) --------
            if fp8_cross and ql is None:
                ql = pre.tile([128, 2, HW], f16, name="ql")
            if qk_bias:
                projs = [(wk_h, wk_l, bk_row, kh, kl),
                         (wq_h, wq_l, bq_row, qh, ql)]
            else:
                projs = [(m2_h, m2_l, None, qh, ql)]   # u = M2^T hn -> qh/ql
            for (w_h, w_l, b_row, out_h, out_l) in projs:
                for mc in range(2):
                    for nh in range(4):
                        n0 = nh * 1024
                        ps_q = psA.tile([128, 1024], f32, name="ps_q", tag="psA")
                        pieces = []
                        for kc in range(2):
                            pieces += [
                                (w_h[:, kc, mc * 128:(mc + 1) * 128], hnh[:, kc]),
                                (w_l[:, kc, mc * 128:(mc + 1) * 128], hnh[:, kc]),
                                (w_h[:, kc, mc * 128:(mc + 1) * 128], hnl[:, kc]),
                            ]
                        for idx, (lhs, rhsrow) in enumerate(pieces):
                            for ns in range(2):
                                j0 = n0 + ns * 512
                                nc.tensor.matmul(ps_q[:, ns * 512:(ns + 1) * 512],
                                                 lhs, rhsrow[:, j0:j0 + 512],
                                                 start=(idx == 0),
                                                 stop=(not qk_bias and idx == len(pieces) - 1))
                        if qk_bias:
                            for hl in range(2):
                                for ns in range(2):
                                    nc.tensor.matmul(ps_q[:, ns * 512:(ns + 1) * 512],
                                                     b_row[:, hl, mc * 128:(mc + 1) * 128],
                                                     ones_row, start=False,
                                                     stop=(hl == 1))
                        nc.scalar.activation(out_h[:, mc, n0:n0 + 1024], ps_q,
                                             mybir.ActivationFunctionType.Copy)
                        nc.vector.tensor_tensor(out=out_l[:, mc, n0:n0 + 1024],
                                                in0=ps_q,
                                                in1=out_h[:, mc, n0:n0 + 1024],
                                                op=mybir.AluOpType.subtract)

            if fp8_cross:
                # product-preserving scales keep the DR result directly
                # accumulable with the fp16 terms in PSUM (no rescale)
                nc.scalar.activation(ul8, ql, mybir.ActivationFunctionType.Copy,
                                     scale=8.0)
                nc.scalar.activation(hh8, hnh, mybir.ActivationFunctionType.Copy,
                                     scale=0.125)

            # -------------- v projection, transposed: vT[i, c] --------------
            for it in range(NT):
                i0 = it * 128
                ps_v = psB.tile([128, C], f32, name="ps_v", tag="psB")
                for kc in range(2):
                    nc.tensor.matmul(ps_v, hnh[:, kc, i0:i0 + 128],
                                     wv_h[:, kc, :], start=(kc == 0),
                                     stop=(not v_bias and kc == 1))
                if v_bias:
                    nc.tensor.matmul(ps_v, ones_row[:, 0:128], bv_row,
                                     start=False, stop=True)
                nc.scalar.activation(vT[:, it, :], ps_v,
                                     mybir.ActivationFunctionType.Copy)

        # ---------------- main attention loop ----------------
        # i-blocks of 512 columns (4 i-tiles) for the AV matmul: N=512 per
        # matmul = 1 psum bank, and the vT stationary operand is loaded once
        # per (jc, mc) per block.
        with tc.tile_pool(name="loop", bufs=2) as lp:
            eT = None
            for t in range(NT):
                i0 = t * 128
                blk, il = t // 4, t % 4
                if il == 0:
                    eT = lp.tile([128, NT, 512], bf16, name="eT", tag="eT", bufs=1)

                # scores for i-tile t: [128, 4096] via 4 psum tiles of 1024;
                # lhsT-major over half-rows: each of the 6 stationary pieces
                # loads once and streams 4 x 512 columns (2 psum tiles).
                sc_sb = lp.tile([128, HW], f32, name="sc_sb", tag="sc")
                rh, rl = (kh, kl) if qk_bias else (hnh, hnl)
                pieces = []
                for kc in range(2):
                    pieces.append((qh[:, kc, i0:i0 + 128], rh[:, kc]))
                    if not fp8_cross:
                        pieces.append((ql[:, kc, i0:i0 + 128], rh[:, kc]))
                    pieces.append((qh[:, kc, i0:i0 + 128], rl[:, kc]))
                for jc in range(4):
                    ps_s = psA.tile([128, 1024], f32, name="ps_s", tag="psA")
                    for idx, (lhs, rhsrow) in enumerate(pieces):
                        for ns in range(2):
                            j0 = jc * 1024 + ns * 512
                            nc.tensor.matmul(ps_s[:, ns * 512:(ns + 1) * 512],
                                             lhs, rhsrow[:, j0:j0 + 512],
                                             start=(idx == 0),
                                             stop=(not fp8_cross and idx == len(pieces) - 1))
                    if fp8_cross:
                        # ul.hh cross-term as one fp8 DoubleRow matmul (K=256
                        # packed via the [Ki,2,dim] interleave; HW-probe-validated)
                        for ns in range(2):
                            j0 = jc * 1024 + ns * 512
                            nc.tensor.matmul(ps_s[:, ns * 512:(ns + 1) * 512],
                                             ul8[:, :, i0:i0 + 128],
                                             hh8[:, :, j0:j0 + 512],
                                             start=False, stop=True,
                                             perf_mode=mybir.MatmulPerfMode.DoubleRow)
                    if fp8_cross and jc % 2 == 1:
                        # fp8 path: ScalarE is co-critical; split copies DVE/ACT
                        nc.vector.tensor_copy(sc_sb[:, jc * 1024:(jc + 1) * 1024],
                                              ps_s)
                    else:
                        nc.scalar.activation(sc_sb[:, jc * 1024:(jc + 1) * 1024],
                                             ps_s, mybir.ActivationFunctionType.Copy)

                e_t = lp.tile([128, HW], bf16, name="e_t", tag="e_t", bufs=3)
                nmax = lp.tile([128, 1], f32, name="nmax", tag="nmax")
                rden = lp.tile([128, 1], f32, name="rden", tag="rden")
                if t < NT - 1:
                    den = lp.tile([128, 1], f32, name="den", tag="den")
                    nc.vector.tensor_reduce(nmax, sc_sb, axis=mybir.AxisListType.X,
                                            op=mybir.AluOpType.max, negate=True)
                    nc.scalar.activation(e_t, sc_sb, mybir.ActivationFunctionType.Exp,
                                         bias=nmax, scale=1.0, accum_out=den)
                    nc.vector.reciprocal(rden, den)
                    nc.vector.tensor_scalar(e_t, e_t, rden, None,
                                            op0=mybir.AluOpType.mult)
                    for tc8 in range(8):
                        nc.sync.dma_start_transpose(
                            eT[:, tc8 * 4:(tc8 + 1) * 4, il * 128:(il + 1) * 128],
                            e_t[:, tc8 * 512:(tc8 + 1) * 512])
                else:
                    # final tile: chunk max/exp/scale/transpose to shorten the
                    # kernel-tail dependency chain
                    cmax = lp.tile([128, 4], f32, name="cmax", tag="cmax")
                    for jc in range(4):
                        nc.vector.tensor_reduce(cmax[:, jc:jc + 1],
                                                sc_sb[:, jc * 1024:(jc + 1) * 1024],
                                                axis=mybir.AxisListType.X,
                                                op=mybir.AluOpType.max)
                    nc.vector.tensor_reduce(nmax, cmax, axis=mybir.AxisListType.X,
                                            op=mybir.AluOpType.max, negate=True)
                    dh = lp.tile([128, 2], f32, name="dh", tag="dh")
                    for hfl in range(2):
                        nc.scalar.activation(e_t[:, hfl * 2048:(hfl + 1) * 2048],
                                             sc_sb[:, hfl * 2048:(hfl + 1) * 2048],
                                             mybir.ActivationFunctionType.Exp,
                                             bias=nmax, scale=1.0,
                                             accum_out=dh[:, hfl:hfl + 1])
                    den2 = lp.tile([128, 1], f32, name="den2", tag="den2")
                    nc.vector.tensor_tensor(out=den2, in0=dh[:, 0:1], in1=dh[:, 1:2],
                                            op=mybir.AluOpType.add)
                    nc.vector.reciprocal(rden, den2)
                    for hfl in range(2):
                        nc.vector.tensor_scalar(e_t[:, hfl * 2048:(hfl + 1) * 2048],
                                                e_t[:, hfl * 2048:(hfl + 1) * 2048],
                                                rden, None, op0=mybir.AluOpType.mult)
                        for tc8 in range(4):
                            c = hfl * 4 + tc8
                            nc.sync.dma_start_transpose(
                                eT[:, c * 4:(c + 1) * 4, il * 128:(il + 1) * 128],
                                e_t[:, c * 512:(c + 1) * 512])

                last_blk = (blk == NT // 4 - 1)
                if il == 2 and last_blk:
                    # partial AV for the final block over its first 3 column
                    # stripes (N=384) so the kernel tail only owes the last
                    # 128-column stripe after the final softmax chain.
                    ps_h = [psB.tile([128, 512], f32, name=f"ps_h{m}", tag="psB")
                            for m in range(2)]
                    for jc in range(NT):
                        for mc in range(2):
                            nc.tensor.matmul(ps_h[mc][:, 0:384],
                                             vT[:, jc, mc * 128:(mc + 1) * 128],
                                             eT[:, jc, 0:384],
                                             start=(jc == 0), stop=(jc == NT - 1))
                if il == 3:
                    # AV for block blk: h[c, i] = sum_j vT[j,c] * eT[j, i]
                    if not last_blk:
                        ps_h = [psB.tile([128, 512], f32, name=f"ps_h{m}", tag="psB")
                                for m in range(2)]
                    for jc in range(NT):
                        for mc in range(2):
                            sl = slice(384, 512) if last_blk else slice(0, 512)
                            nc.tensor.matmul(ps_h[mc][:, sl],
                                             vT[:, jc, mc * 128:(mc + 1) * 128],
                                             eT[:, jc, sl],
                                             start=(jc == 0), stop=(jc == NT - 1))
                    for mc in range(2):
                        o_sb = lp.tile([128, 512], f32, name="o_sb", tag="o_sb")
                        nc.vector.tensor_tensor(out=o_sb, in0=ps_h[mc],
                                                in1=proj_sb[:, mc, blk * 512:(blk + 1) * 512],
                                                op=mybir.AluOpType.add)
                        nc.sync.dma_start(
                            d_out[mc * 128:(mc + 1) * 128, blk * 512:(blk + 1) * 512],
                            o_sb)

    _dedup_ldweights(nc)
    nc.compile()
    return nc


def _build_program_fast(v_bias, fp8_terms=2):
    """Optimized zero-qk-bias path.

    Scores s = u^T hn with u = M2^T hn (M2 = (Wq*-0.5C)^T Wk).  fp16 hi/lo
    split of u and hn gives s = uh^T hh (fp16, exact-grade) plus two small
    cross terms ul^T hh and uh^T hl, each run as ONE fp8e4 DoubleRow matmul
    with K=256 packed via the [Ki,2,dim] interleave at 0.5 cyc/row (product-
    preserving scale pairs keep PSUM accumulation direct).  Softmax is fully
    chunked: per 1024-col chunk, DVE takes -max from PSUM, ACT exps straight
    out of PSUM (bias=-chunk-max, accum_out=chunk denom) into bf16 SBUF, and
    a per-chunk fixup scale e^{m_jc-m}/den folds the global max AND the
    normalization into one small multiply -- no fp32 score copy to SBUF at
    all.
    """
    import os as _os
    # InstTensorTensorReduce crashes real TRN2 (NRT_EXEC_UNIT_UNRECOVERABLE,
    # HW-bisected 2026-08-07) even though CoreSim/TimelineSim accept it --
    # always use the separate tensor_tensor + tensor_reduce pair instead.
    no_ttr = True
    exp_sbuf = bool(_os.environ.get("ATTN_EXP_SBUF"))
    tr512 = bool(_os.environ.get("ATTN_TR512"))
    no_negscale = bool(_os.environ.get("ATTN_NO_NEGSCALE"))

    nc = bacc.Bacc("TRN2", target_bir_lowering=False, debug=False)

    def din(name, shape, dt=f32):
        return nc.dram_tensor(name, shape, dt, kind="ExternalInput").ap()

    d_x = din("x", (C, HW))
    d_m2_h, d_m2_l = din("m2_h", (C, C), f16), din("m2_l", (C, C), f16)
    d_wv_h = din("wv_h", (C, C), f16)
    d_wo_h = din("wo_h", (C, C), f16)
    d_gamma, d_beta = din("gamma", (C,)), din("beta", (C,))
    d_bv = din("bv", (1, C), f16)
    d_bo16 = din("bo16", (1, C), f16)
    d_gmat = din("gmat", (128, 128))
    d_out = nc.dram_tensor("out", (C, HW), f32, kind="ExternalOutput").ap()

    DR = mybir.MatmulPerfMode.DoubleRow
    f32r = mybir.dt.float32r

    with tile.TileContext(nc) as tc, ExitStack() as ctx:
        main = ctx.enter_context(tc.tile_pool(name="main", bufs=1))
        psA = ctx.enter_context(tc.tile_pool(name="psA", bufs=3, space="PSUM"))
        psB = ctx.enter_context(tc.tile_pool(name="psB", bufs=2, space="PSUM"))

        # ---------------- persistent tiles ----------------
        wv_h = main.tile([128, 2, C], f16, name="wv_h")
        m2_h = main.tile([128, 2, C], f16, name="m2_h")
        m2_l = main.tile([128, 2, C], f16, name="m2_l")
        for tl_, d in [(m2_h, d_m2_h), (m2_l, d_m2_l), (wv_h, d_wv_h)]:
            nc.sync.dma_start(tl_, d.rearrange("(kc kl) m -> kl kc m", kl=128))

        qh = main.tile([128, 2, HW], f16, name="qh")
        hnh = main.tile([128, 2, HW], f16, name="hnh")
        ul8 = main.tile([128, 2, HW], f8e4, name="ul8")
        hh8 = main.tile([128, 2, HW], f8e4, name="hh8")
        if fp8_terms == 2:
            uh8 = main.tile([128, 2, HW], f8e4, name="uh8")
            hl8 = main.tile([128, 2, HW], f8e4, name="hl8")
            # fp8 copies of M2 for the u-projection cross terms, scale-paired
            # with the existing hh8 (hn/8) and hl8 (hn_lo*256)
            m2l8 = main.tile([128, 2, C], f8e4, name="m2l8")
            m2h8 = main.tile([128, 2, C], f8e4, name="m2h8")
        vT = main.tile([128, NT, C], bf16, name="vT")
        # xh16: f16 copy of x for the out-projection Wo@x, which runs folded
        # into each block's AV PSUM accumulation (no separate proj pass, no
        # proj_sb; f16 is ample for this post-softmax path).  NOTE fp32r
        # matmuls straight off the DMA-landed fp32 x are rejected by the BIR
        # verifier (operands must be rounded to fp32r by a compute engine),
        # so the f16 cast path is the cheap legal option.
        xh16 = main.tile([128, 2, HW], f16, name="xh16")
        wo_h = main.tile([128, 2, C], f16, name="wo_h")
        nc.gpsimd.dma_start(wo_h, d_wo_h.rearrange("(kc kl) m -> kl kc m", kl=128))

        gamma_sb = main.tile([128, 2], f32, name="gamma_sb")
        beta_sb = main.tile([128, 2], f32, name="beta_sb")
        nc.sync.dma_start(gamma_sb, d_gamma.rearrange("(t p) -> p t", p=128))
        nc.sync.dma_start(beta_sb, d_beta.rearrange("(t p) -> p t", p=128))
        bo_row = main.tile([1, C], f16, name="bo_row")
        nc.sync.dma_start(bo_row, d_bo16)
        bv_row = main.tile([1, C], f16, name="bv_row")
        nc.sync.dma_start(bv_row, d_bv)
        ones_row = main.tile([1, 512], f16, name="ones_row")
        nc.vector.memset(ones_row, 1.0)
        eps_t = main.tile([128, 2], f32, name="eps_t")
        nc.vector.memset(eps_t, EPS)

        with tc.tile_pool(name="pre", bufs=1) as pre:
            gmat = pre.tile([128, 128], f32, name="gmat")
            nc.sync.dma_start(gmat, d_gmat)
            # p-state warmup: the PE would otherwise idle ~20us during the x
            # load and then pay the 2-3.7x cold-clock penalty on the first
            # real matmuls.  Chew on the already-landed m2 weights back to
            # back so the ramp (3us continuous -> full clock) completes
            # before the projections start.
            n_warm = int(_os.environ.get("ATTN_WARM", "0"))
            for w in range(n_warm):
                ps_w = psA.tile([128, 1024], f32, name="ps_w", tag="psA")
                nc.tensor.matmul(ps_w[:, 0:512], m2_h[:, 0, 0:128],
                                 m2_h.rearrange("p a b -> p (a b)")[:, 0:512],
                                 start=True, stop=True)
            if fp8_terms == 2:
                nc.scalar.activation(m2l8, m2_l,
                                     mybir.ActivationFunctionType.Copy,
                                     scale=8.0)
                nc.scalar.activation(m2h8, m2_h,
                                     mybir.ActivationFunctionType.Copy,
                                     scale=1.0 / 256.0)

            # -------------- load x, GroupNorm stats --------------
            x_sb = pre.tile([128, 2, HW], f32, name="x_sb")
            d_xv = d_x.rearrange("(t p) n -> p t n", p=128)
            for c8 in range(8):
                for t in range(2):
                    n0 = c8 * 512
                    eng = nc.scalar if c8 == 0 else nc.sync
                    eng.dma_start(x_sb[:, t, n0:n0 + 512],
                                  d_xv[:, t, n0:n0 + 512])

            # f16 copy of x chunks as the x DMAs land (for the fused out-proj)
            for c8 in range(8):
                for t in range(2):
                    n0 = c8 * 512
                    nc.scalar.activation(xh16[:, t, n0:n0 + 512],
                                         x_sb[:, t, n0:n0 + 512],
                                         mybir.ActivationFunctionType.Copy)

            stats = pre.tile([128, 2, 8, 6], f32, name="stats")
            stats_src = xh16 if _os.environ.get("ATTN_STATS16") else x_sb
            for t in range(2):
                xv = stats_src[:, t, :].rearrange("p (s n) -> p s n", n=512)
                for s in range(8):
                    nc.vector.bn_stats(stats[:, t, s, :], xv[:, s, :])
            mv = pre.tile([128, 2, 2], f32, name="mv")
            a_sc = pre.tile([128, 2], f32, name="a_sc")
            b_sc = pre.tile([128, 2], f32, name="b_sc")
            stat2 = pre.tile([128, 2, 2], f32, name="stat2")
            gstat = pre.tile([128, 2, 2], f32, name="gstat")
            gvar = pre.tile([128, 2], f32, name="gvar")
            seps = pre.tile([128, 2], f32, name="seps")
            rstd = pre.tile([128, 2], f32, name="rstd")
            tmp = pre.tile([128, 2], f32, name="tmp")
            for t in range(2):
                nc.vector.bn_aggr(mv[:, t, :], stats[:, t, :, :])
                nc.vector.tensor_tensor(out=stat2[:, t, 1:2], in0=mv[:, t, 0:1],
                                        in1=mv[:, t, 0:1], op=mybir.AluOpType.mult)
                nc.vector.tensor_tensor(out=stat2[:, t, 1:2], in0=stat2[:, t, 1:2],
                                        in1=mv[:, t, 1:2], op=mybir.AluOpType.add)
                # mean copy on DVE: a Pool round trip here puts Q7 launch
                # + two sem hops into the stats critical chain
                nc.vector.tensor_copy(stat2[:, t, 0:1], mv[:, t, 0:1])
                ps_g = psB.tile([128, 2], f32, name="ps_g", tag="psB")
                nc.tensor.matmul(ps_g, gmat, stat2[:, t, :], start=True, stop=True)
                nc.vector.tensor_copy(gstat[:, t, :], ps_g)
                gm = gstat[:, t, 0:1]
                nc.vector.tensor_tensor(out=gvar[:, t:t + 1], in0=gm, in1=gm,
                                        op=mybir.AluOpType.mult)
                nc.vector.tensor_tensor(out=gvar[:, t:t + 1], in0=gstat[:, t, 1:2],
                                        in1=gvar[:, t:t + 1],
                                        op=mybir.AluOpType.subtract)
                nc.vector.tensor_tensor(out=seps[:, t:t + 1], in0=gvar[:, t:t + 1],
                                        in1=eps_t[:, t:t + 1], op=mybir.AluOpType.add)
                nc.scalar.activation(rstd[:, t:t + 1], seps[:, t:t + 1],
                                     mybir.ActivationFunctionType.Sqrt)
                nc.vector.reciprocal(rstd[:, t:t + 1], rstd[:, t:t + 1])
                for _ in range(int(_os.environ.get("ATTN_NR", "0"))):
                    nc.vector.tensor_tensor(out=tmp[:, t:t + 1], in0=rstd[:, t:t + 1],
                                            in1=rstd[:, t:t + 1], op=mybir.AluOpType.mult)
                    nc.vector.tensor_tensor(out=tmp[:, t:t + 1], in0=tmp[:, t:t + 1],
                                            in1=seps[:, t:t + 1], op=mybir.AluOpType.mult)
                    nc.vector.tensor_scalar(tmp[:, t:t + 1], tmp[:, t:t + 1], -0.5, 1.5,
                                            op0=mybir.AluOpType.mult,
                                            op1=mybir.AluOpType.add)
                    nc.vector.tensor_tensor(out=rstd[:, t:t + 1], in0=rstd[:, t:t + 1],
                                            in1=tmp[:, t:t + 1], op=mybir.AluOpType.mult)
                nc.vector.tensor_tensor(out=a_sc[:, t:t + 1], in0=rstd[:, t:t + 1],
                                        in1=gamma_sb[:, t:t + 1], op=mybir.AluOpType.mult)
                nc.vector.tensor_tensor(out=b_sc[:, t:t + 1], in0=gstat[:, t, 0:1],
                                        in1=a_sc[:, t:t + 1], op=mybir.AluOpType.mult)
                nc.vector.tensor_tensor(out=b_sc[:, t:t + 1], in0=beta_sb[:, t:t + 1],
                                        in1=b_sc[:, t:t + 1], op=mybir.AluOpType.subtract)

            # -------------- hn (scratch) and fp16 hi/lo split --------------
            # hh8 on ACT; hl8 on the otherwise-idle Pool engine.  fp8 copies
            # are chunk-interleaved so they pipeline behind the DVE hn chunks
            hnl = (pre if fp8_terms == 2 else main).tile(
                [128, 2, HW], f16, name="hnl")
            for c4 in range(4):
                for t in range(2):
                    n0 = c4 * 1024
                    sl = (slice(None), t, slice(n0, n0 + 1024))
                    scr = pre.tile([128, 1024], f32, name="hnscr",
                                   tag="hnscr", bufs=2)
                    nc.vector.tensor_scalar(scr, x_sb[sl],
                                            a_sc[:, t:t + 1], b_sc[:, t:t + 1],
                                            op0=mybir.AluOpType.mult,
                                            op1=mybir.AluOpType.add)
                    # hnh extract stays on DVE: it sits on the serial hn
                    # spine (hnl needs it) and a cross-engine hop here costs
                    # more latency than DVE throughput
                    nc.vector.tensor_copy(hnh[sl], scr)
                    nc.vector.tensor_tensor(out=hnl[sl], in0=scr, in1=hnh[sl],
                                            op=mybir.AluOpType.subtract)
                    nc.scalar.activation(hh8[sl], hnh[sl],
                                         mybir.ActivationFunctionType.Copy,
                                         scale=0.125)
                    if fp8_terms == 2:
                        nc.gpsimd.tensor_scalar(hl8[sl], hnl[sl], 256.0, None,
                                                op0=mybir.AluOpType.mult)

            # ---- u projection (fp16-split, exact-grade) + v projection ----
            # interleaved per 1024-column chunk so the DVE queue (ql extract,
            # vT copies) drains alongside the PE stream and the first score
            # maxes aren't stuck behind 32 queued vT copies
            ql = pre.tile([128, 2, HW], f16, name="ql")
            for nh in range(4):
                n0 = nh * 1024
                for mc in range(2):
                    ps_q = psA.tile([128, 1024], f32, name="ps_q", tag="psA")
                    if fp8_terms == 2:
                        # hi-hi fp16 + both cross terms as fp8 DoubleRow
                        # (K=256 packed), reusing the scores' hh8/hl8
                        pieces = [
                            (m2_h[:, 0, mc * 128:(mc + 1) * 128], hnh[:, 0], None),
                            (m2_h[:, 1, mc * 128:(mc + 1) * 128], hnh[:, 1], None),
                            (m2l8[:, :, mc * 128:(mc + 1) * 128], hh8, DR),
                            (m2h8[:, :, mc * 128:(mc + 1) * 128], hl8, DR),
                        ]
                    else:
                        pieces = []
                        for kc in range(2):
                            pieces += [
                                (m2_h[:, kc, mc * 128:(mc + 1) * 128], hnh[:, kc], None),
                                (m2_l[:, kc, mc * 128:(mc + 1) * 128], hnh[:, kc], None),
                                (m2_h[:, kc, mc * 128:(mc + 1) * 128], hnl[:, kc], None),
                            ]
                    for idx, (lhs, rhsrow, pm) in enumerate(pieces):
                        for ns in range(2):
                            j0 = n0 + ns * 512
                            psl = ps_q[:, ns * 512:(ns + 1) * 512]
                            st, sp = (idx == 0), (idx == len(pieces) - 1)
                            if pm is None:
                                nc.tensor.matmul(psl, lhs,
                                                 rhsrow[:, j0:j0 + 512],
                                                 start=st, stop=sp)
                            else:
                                nc.tensor.matmul(psl, lhs,
                                                 rhsrow[:, :, j0:j0 + 512],
                                                 start=st, stop=sp,
                                                 perf_mode=pm)
                    nc.scalar.activation(qh[:, mc, n0:n0 + 1024], ps_q,
                                         mybir.ActivationFunctionType.Copy)
                    nc.vector.tensor_tensor(out=ql[:, mc, n0:n0 + 1024],
                                            in0=ps_q,
                                            in1=qh[:, mc, n0:n0 + 1024],
                                            op=mybir.AluOpType.subtract)
                    # chunked fp8 copies pipeline behind the projection
                    nc.scalar.activation(ul8[:, mc, n0:n0 + 1024],
                                         ql[:, mc, n0:n0 + 1024],
                                         mybir.ActivationFunctionType.Copy,
                                         scale=8.0)
                    if fp8_terms == 2:
                        nc.gpsimd.tensor_scalar(uh8[:, mc, n0:n0 + 1024],
                                                qh[:, mc, n0:n0 + 1024],
                                                1.0 / 256.0, None,
                                                op0=mybir.AluOpType.mult)
                pass  # v-projection is emitted inside the main loop (after
                # scores tile 0): vT is first read by AV(0) at tile 4, so
                # deferring it starts the score loop one pipeline stage sooner

        # ---------------- main attention loop ----------------
        # Two levels of software pipelining keep every engine streaming:
        #  * tile level: the softmax tail of tile t-1 (global-max fixup chain,
        #    which round-trips DVE->ACT->DVE, the fixup scales, and the eT
        #    transposes) is emitted in the middle of tile t's chunk stream, so
        #    DVE's in-order queue overlaps the ACT round trip with tile t's
        #    chunk maxes instead of head-of-line blocking on it.
        #  * block level: AV for block b is emitted AFTER the score matmuls of
        #    tile 4b+4 so the PE has score work while block b's tail drains
        #    (eT is double-buffered to allow it).
        with tc.tile_pool(name="loop", bufs=2) as lp:
            eT_blk = {}
            state = {}
            ps_h = None

            tail_a = {}

            def emit_tail_a(t):
                """Tail part A for tile t: everything up to (and including)
                the DVE->ACT round trip for e^{m_jc - m}.  Emitted separately
                from part B so the next chunk's max can run on DVE while the
                ACT hop is in flight."""
                cmax, denj, e_t = state.pop(t)
                nm = lp.tile([128, 1], f32, name="nm", tag="nm")
                fsc = lp.tile([128, 4], f32, name="fsc", tag="fsc")
                nc.vector.tensor_reduce(nm, cmax, axis=mybir.AxisListType.X,
                                        op=mybir.AluOpType.min)  # nm = -m
                nc.vector.tensor_scalar(fsc, cmax, nm, None,
                                        op0=mybir.AluOpType.subtract)  # m - m_jc
                nc.scalar.activation(fsc, fsc,
                                     mybir.ActivationFunctionType.Exp,
                                     scale=-1.0)  # e^{m_jc - m}
                tail_a[t] = (denj, e_t, fsc)

            def emit_tail_b(t):
                """Tail part B: denominator combine, fixup scales, transposes."""
                denj, e_t, fsc = tail_a.pop(t)
                eT = eT_blk[t // 4]
                il = t % 4
                dsum = lp.tile([128, 4], f32, name="dsum", tag="dsum")
                dden = lp.tile([128, 1], f32, name="dden", tag="dden")
                rden = lp.tile([128, 1], f32, name="rden", tag="rden")
                g = lp.tile([128, 4], f32, name="g", tag="g")
                if no_ttr:
                    nc.vector.tensor_tensor(out=dsum, in0=denj, in1=fsc,
                                            op=mybir.AluOpType.mult)
                    nc.vector.tensor_reduce(dden, dsum,
                                            axis=mybir.AxisListType.X,
                                            op=mybir.AluOpType.add)
                else:
                    nc.vector.tensor_tensor_reduce(out=dsum, in0=denj, in1=fsc,
                                                   scale=1.0, scalar=0.0,
                                                   op0=mybir.AluOpType.mult,
                                                   op1=mybir.AluOpType.add,
                                                   accum_out=dden)
                nc.vector.reciprocal(rden, dden)
                nc.vector.tensor_scalar(g, fsc, rden, None,
                                        op0=mybir.AluOpType.mult)
                for jc in range(4):
                    # first two fixups ride the idle Pool engine (their
                    # transposes have slack); the last two stay on DVE so the
                    # block's eT completes before the delayed AV needs it
                    fix_pool = int(_os.environ.get("ATTN_FIXPOOL", "0"))
                    if jc < fix_pool:
                        nc.gpsimd.tensor_scalar(e_t[:, jc * 1024:(jc + 1) * 1024],
                                                e_t[:, jc * 1024:(jc + 1) * 1024],
                                                g[:, jc:jc + 1], None,
                                                op0=mybir.AluOpType.mult)
                    else:
                        nc.vector.tensor_scalar(e_t[:, jc * 1024:(jc + 1) * 1024],
                                                e_t[:, jc * 1024:(jc + 1) * 1024],
                                                g[:, jc:jc + 1], None,
                                                op0=mybir.AluOpType.mult)
                    if tr512:
                        for h2 in range(2):
                            cc = jc * 2 + h2
                            nc.sync.dma_start_transpose(
                                eT[:, cc * 4:(cc + 1) * 4, il * 128:(il + 1) * 128],
                                e_t[:, cc * 512:(cc + 1) * 512])
                    else:
                        nc.sync.dma_start_transpose(
                            eT[:, jc * 8:(jc + 1) * 8, il * 128:(il + 1) * 128],
                            e_t[:, jc * 1024:(jc + 1) * 1024])

            def emit_tail(t):
                emit_tail_a(t)
                emit_tail_b(t)

            def emit_av_open(b, sl):
                # out-projection (Wo@x + bo) opens each stripe's PSUM group;
                # it depends only on xh16, so at block boundaries it gives the
                # PE dep-free work while DVE/ACT drain the chunk backlog
                g0 = b * 512 + sl.start
                gn = sl.stop - sl.start
                for mc in range(2):
                    for kc in range(2):
                        nc.tensor.matmul(ps_h[mc][:, sl],
                                         wo_h[:, kc, mc * 128:(mc + 1) * 128],
                                         xh16[:, kc, g0:g0 + gn],
                                         start=(kc == 0), stop=False)
                    nc.tensor.matmul(ps_h[mc][:, sl],
                                     bo_row[:, mc * 128:(mc + 1) * 128],
                                     ones_row[:, 0:gn],
                                     start=False, stop=False)

            def emit_av_body(b, sl):
                eTb = eT_blk[b]
                for jc in range(NT):
                    for mc in range(2):
                        nc.tensor.matmul(ps_h[mc][:, sl],
                                         vT[:, jc, mc * 128:(mc + 1) * 128],
                                         eTb[:, jc, sl],
                                         start=False, stop=(jc == NT - 1))

            def emit_av(b, sl):
                emit_av_open(b, sl)
                emit_av_body(b, sl)

            def emit_out(b, sl=slice(0, 512)):
                g0 = b * 512 + sl.start
                gn = sl.stop - sl.start
                for mc in range(2):
                    o_sb = lp.tile([128, 512], f32, name="o_sb", tag="o_sb")
                    nc.vector.tensor_copy(o_sb[:, 0:gn], ps_h[mc][:, sl])
                    nc.sync.dma_start(
                        d_out[mc * 128:(mc + 1) * 128, g0:g0 + gn],
                        o_sb[:, 0:gn])

            for t in range(NT):
                i0 = t * 128
                blk, il = t // 4, t % 4
                if t == int(_os.environ.get("ATTN_VPOS", "3")):
                    # deferred v-projection: runs on PE after scores tile 0,
                    # well before its first reader AV(0) at tile 4
                    for it in range(NT):
                        iv = it * 128
                        ps_v = psB.tile([128, C], f32, name="ps_v", tag="psB")
                        for kc in range(2):
                            nc.tensor.matmul(ps_v, hnh[:, kc, iv:iv + 128],
                                             wv_h[:, kc, :], start=(kc == 0),
                                             stop=(not v_bias and kc == 1))
                        if v_bias:
                            nc.tensor.matmul(ps_v, ones_row[:, 0:128], bv_row,
                                             start=False, stop=True)
                        if it % 2 == 0:
                            nc.vector.tensor_copy(vT[:, it, :], ps_v)
                        else:
                            nc.scalar.activation(vT[:, it, :], ps_v,
                                                 mybir.ActivationFunctionType.Copy)
                if il == 0:
                    eT_blk[blk] = lp.tile([128, NT, 512], bf16, name="eT",
                                          tag="eT", bufs=2)
                    eT_blk.pop(blk - 2, None)

                e_t = lp.tile([128, HW], bf16, name="e_t", tag="e_t", bufs=4)
                cmax = lp.tile([128, 4], f32, name="cmax", tag="cmax")
                denj = lp.tile([128, 4], f32, name="denj", tag="denj")
                state[t] = (cmax, denj, e_t)

                boundary_av = (il == 0 and blk >= 1)

                pieces = [
                    (qh[:, 0, i0:i0 + 128], hnh[:, 0], None),
                    (qh[:, 1, i0:i0 + 128], hnh[:, 1], None),
                ]
                if fp8_terms < 2:
                    pieces += [(qh[:, 0, i0:i0 + 128], hnl[:, 0], None),
                               (qh[:, 1, i0:i0 + 128], hnl[:, 1], None)]
                    # re-order so each qh half loads once (ldweights dedup)
                    pieces = [pieces[0], pieces[2], pieces[1], pieces[3]]
                pieces.append((ul8[:, :, i0:i0 + 128], hh8, DR))
                if fp8_terms == 2:
                    pieces.append((uh8[:, :, i0:i0 + 128], hl8, DR))

                for jc in range(4):
                    ps_s = psA.tile([128, 1024], f32, name="ps_s", tag="psA")
                    for idx, (lhs, rhs, pm) in enumerate(pieces):
                        for ns in range(2):
                            j0 = jc * 1024 + ns * 512
                            psl = ps_s[:, ns * 512:(ns + 1) * 512]
                            st, sp = (idx == 0), (idx == len(pieces) - 1)
                            if pm is None:
                                nc.tensor.matmul(psl, lhs, rhs[:, j0:j0 + 512],
                                                 start=st, stop=sp)
                            else:
                                nc.tensor.matmul(psl, lhs, rhs[:, :, j0:j0 + 512],
                                                 start=st, stop=sp, perf_mode=pm)
                    if exp_sbuf:
                        sc_sb = lp.tile([128, 1024], f32, name="sc_sb",
                                        tag="sc_sb", bufs=3)
                        nc.scalar.activation(sc_sb, ps_s,
                                             mybir.ActivationFunctionType.Copy)
                        src = sc_sb
                    else:
                        src = ps_s
                    nc.vector.tensor_reduce(cmax[:, jc:jc + 1], src,
                                            axis=mybir.AxisListType.X,
                                            op=mybir.AluOpType.max, negate=True)
                    nc.scalar.activation(e_t[:, jc * 1024:(jc + 1) * 1024], src,
                                         mybir.ActivationFunctionType.Exp,
                                         bias=cmax[:, jc:jc + 1], scale=1.0,
                                         accum_out=denj[:, jc:jc + 1])
                    if jc == 1 and t >= 1 and (t - 1) in state:
                        emit_tail(t - 1)

                last_blk = (blk == NT // 4 - 1)
                if boundary_av:
                    ps_h = [psB.tile([128, 512], f32, name=f"ps_h{m}", tag="psB")
                            for m in range(2)]
                    emit_av(blk - 1, slice(0, 512))
                    emit_out(blk - 1)
                if last_blk and il == 2:
                    # final block: AV in stripes, each gated only on tails
                    # that are already emitted, so the PE never waits; each
                    # stripe's output ships as soon as its group stops
                    ps_h = [psB.tile([128, 512], f32, name=f"ps_h{m}", tag="psB")
                            for m in range(2)]
                    emit_av(blk, slice(0, 256))   # tiles 28,29 tails done
                if last_blk and il == 3:
                    emit_av(blk, slice(256, 384))  # tile 30 tail done (jc1)
                    emit_tail(t)
                    emit_av(blk, slice(384, 512))
                    emit_out(blk)

    _dedup_ldweights(nc)
    nc.compile()
    return nc


def _dedup_ldweights(nc):
    """Remove back-to-back InstLdweights that reload the identical stationary
    operand on the PE stream (tile splits every matmul into ldweights+matmult,
    even when consecutive matmuls share weights). Any sync info on a removed
    load is merged into the following kept PE instruction."""
    import concourse.mybir as mybir_m

    for f in nc.m.functions:
        for blk in f.blocks:
            insts = blk.instructions
            last_key = None
            pending_waits = []
            pending_updates = []
            keep = []
            removed = 0
            for inst in insts:
                tn = type(inst).__name__
                eng = str(inst.engine)
                if "PE" not in eng:
                    keep.append(inst)
                    continue
                if tn == "InstLdweights":
                    a = inst.ins[0]
                    key = (getattr(a, "memref", None), getattr(a, "offset", None),
                           str(getattr(a, "ap", None)), str(getattr(a, "dtype", None)))
                    if key == last_key:
                        si = inst.sync_info
                        if si is not None:
                            pending_waits += list(si.on_wait)
                            pending_updates += list(si.on_update)
                        removed += 1
                        continue
                    last_key = key
                elif tn == "InstMatmult":
                    # fp32/fp32r matmuls self-load their weights (no separate
                    # InstLdweights), clobbering the PE array state
                    d = str(getattr(inst.ins[0], "dtype", ""))
                    if "float32" in d:
                        last_key = None
                else:
                    # unknown PE instruction: weights state no longer certain
                    last_key = None
                if (pending_waits or pending_updates):
                    si = inst.sync_info
                    if si is None:
                        inst.sync_info = mybir_m.SyncInfo(
                            on_wait=pending_waits, on_update=pending_updates)
                    else:
                        inst.sync_info = mybir_m.SyncInfo(
                            on_wait=list(si.on_wait) + pending_waits,
                            on_update=list(si.on_update) + pending_updates)
                    pending_waits, pending_updates = [], []
                keep.append(inst)
            if removed:
                while len(blk.instructions):
                    blk.instructions.pop()
                for inst in keep:
                    blk.instructions.append(inst)


def _get_program(qk_bias=True, v_bias=True, fp8_cross=False):
    key = (qk_bias, v_bias, fp8_cross)
    if key not in _PROGRAMS:
        _PROGRAMS[key] = _build_program(qk_bias, v_bias, fp8_cross)
    return _PROGRAMS[key]


def _get_program_fast(v_bias=False, fp8_terms=2):
    import os as _os
    knobs = tuple(bool(_os.environ.get(k)) for k in
                  ("ATTN_NO_TTR", "ATTN_EXP_SBUF", "ATTN_TR512",
                   "ATTN_NO_NEGSCALE"))
    key = ("fast", v_bias, fp8_terms, knobs)
    if key not in _PROGRAMS:
        _PROGRAMS[key] = _build_program_fast(v_bias, fp8_terms)
    return _PROGRAMS[key]


def kernel(x, norm_gamma, norm_beta, Wq, bq, Wk, bk, Wv, bv, Wo, bo):
    x = np.ascontiguousarray(np.asarray(x, np.float32))
    assert x.shape == (B, C, H, W)

    def _bias_hl(b32):
        h = b32.astype(np.float16)
        l = (b32 - h.astype(np.float32)).astype(np.float16)
        return np.stack([h, l]).reshape(1, 2, C)

    def split16(w):
        h = w.astype(np.float16)
        l = (w - h.astype(np.float32)).astype(np.float16)
        return h, l

    scale = -0.5 * C
    wq_t = np.ascontiguousarray((np.asarray(Wq, np.float32) * scale).T)
    wk_t = np.ascontiguousarray(np.asarray(Wk, np.float32).T)
    wv_t = np.ascontiguousarray(np.asarray(Wv, np.float32).T)
    wo_t = np.ascontiguousarray(np.asarray(Wo, np.float32).T)
    wq_h, wq_l = split16(wq_t)
    wk_h, wk_l = split16(wk_t)
    # exact bilinear fold for the zero-bias fast path: s = hn^T M2 hn
    m2 = ((np.asarray(Wq, np.float64) * scale).T @ np.asarray(Wk, np.float64))
    m2_h = m2.astype(np.float16)
    m2_l = (m2 - m2_h.astype(np.float64)).astype(np.float16)
    wv_h = wv_t.astype(np.float16)
    wo_h = wo_t.astype(np.float16)

    gmat = np.zeros((128, 128), np.float32)
    for g in range(128 // GS):
        gmat[g * GS:(g + 1) * GS, g * GS:(g + 1) * GS] = 1.0 / GS

    common = {
        "wq_h": wq_h, "wq_l": wq_l, "wk_h": wk_h, "wk_l": wk_l,
        "wv_h": wv_h, "wo_h": wo_h, "wo32": wo_t,
        "m2_h": np.ascontiguousarray(m2_h), "m2_l": np.ascontiguousarray(m2_l),
        "gamma": np.asarray(norm_gamma, np.float32),
        "beta": np.asarray(norm_beta, np.float32),
        "bq": _bias_hl(np.asarray(bq, np.float32) * scale),
        "bk": _bias_hl(np.asarray(bk, np.float32)),
        "bv": np.asarray(bv, np.float32).astype(np.float16).reshape(1, C),
        "bo": np.asarray(bo, np.float32),
        "bo16": np.asarray(bo, np.float32).astype(np.float16).reshape(1, C),
        "gmat": gmat,
    }
    in_maps = [dict(common, x=x[c].reshape(C, HW)) for c in range(B)]

    qk_bias = bool(np.any(np.asarray(bq)) or np.any(np.asarray(bk)))
    v_bias = bool(np.any(np.asarray(bv)))
    import os as _os
    if qk_bias:
        nc = _get_program(qk_bias, v_bias, False)
    else:
        terms = int(_os.environ.get("ATTN_FP8_TERMS", "2"))
        if terms == 0:
            nc = _get_program(False, v_bias, False)
        else:
            nc = _get_program_fast(v_bias, terms)
    global _LAST_PROGRAM
    _LAST_PROGRAM = nc
    import os
    trace = bool(os.environ.get("ATTN_TRACE"))
    res = run_bass_kernel_spmd(nc, in_maps, core_ids=list(range(B)),
                               trace=trace,
                               tmpdir=os.environ.get("ATTN_TRACE_DIR") or None)
    global _LAST_EXEC_NS
    _LAST_EXEC_NS = res.exec_time_ns
    out = np.stack([res.results[c]["out"].reshape(C, H, W) for c in range(B)])
    return out.astype(np.float32)


_LAST_EXEC_NS = None
_LAST_PROGRAM = None


if __name__ == "__main__":
    rng = np.random.default_rng(0)
    ins = {
        "x": rng.standard_normal((B, C, H, W)).astype(np.float32),
        "norm_gamma": np.ones(C, np.float32),
        "norm_beta": np.zeros(C, np.float32),
        "Wq": (rng.standard_normal((C, C)) / 16).astype(np.float32),
        "bq": np.zeros(C, np.float32),
        "Wk": (rng.standard_normal((C, C)) / 16).astype(np.float32),
        "bk": np.zeros(C, np.float32),
        "Wv": (rng.standard_normal((C, C)) / 16).astype(np.float32),
        "bv": np.zeros(C, np.float32),
        "Wo": (rng.standard_normal((C, C)) / 16).astype(np.float32),
        "bo": np.zeros(C, np.float32),
    }
    o = kernel(**ins)
    print("kernel ran, out shape", o.shape, "absmax", np.abs(o).max())



# revision 99
# speedup vs baseline: 1.0011x; 1.0011x over previous
"""AttentionBlock (GroupNorm + single-head 4096x4096 attention + out-proj) on 8
Trainium2 NeuronCores, data-parallel over batch (B=8 -> 1 image per core).

Numerics: scores q^T k must be fp32-grade (softmax over scores with sigma~2000
behaves like an argmax; bf16/fp32r-level score error flips argmaxes and fails).
Strategy: fp16 hi/lo split => q^T k = qh^T kh + ql^T kh + qh^T kl (3 matmuls at
1 cyc/row beats native fp32's 4 cyc/row). q,k projections likewise fp16-split.
The post-softmax path (e, attn, v, AV matmul) is bf16-safe. exp runs on ScalarE
with per-row bias=-max and accum_out giving the denominator for free; the
attention matrix is transposed for the AV matmul with DMA-xbar transposes.
"""
import sys
if "/opt/trn_rl_repo" not in sys.path:
    sys.path.insert(0, "/opt/trn_rl_repo")

from contextlib import ExitStack

import numpy as np

import concourse.bacc as bacc
import concourse.tile as tile
from concourse import mybir
from concourse.bass_utils import run_bass_kernel_spmd

B, C, H, W = 8, 256, 64, 64
HW = H * W            # 4096
G = 32                # groups
GS = C // G           # 8 channels / group
EPS = 1e-5
NT = HW // 128        # 32 i-tiles
NB = NT // 2          # 16 AV blocks of 256 columns

f32 = mybir.dt.float32
f8e4 = mybir.dt.float8e4
f16 = mybir.dt.float16
bf16 = mybir.dt.bfloat16

_PROGRAMS = {}


def _build_program(qk_bias, v_bias, fp8_cross=False):
    nc = bacc.Bacc("TRN2", target_bir_lowering=False, debug=False)

    def din(name, shape, dt=f32):
        return nc.dram_tensor(name, shape, dt, kind="ExternalInput").ap()

    d_x = din("x", (C, HW))
    # weights pre-transposed to [cin, cout]; q side pre-scaled by -0.5*C
    if qk_bias:
        d_wq_h, d_wq_l = din("wq_h", (C, C), f16), din("wq_l", (C, C), f16)
        d_wk_h, d_wk_l = din("wk_h", (C, C), f16), din("wk_l", (C, C), f16)
    else:
        # zero q/k bias: scores fold to u^T hn with u = M2^T hn, M2 = Wq'^T Wk
        d_m2_h, d_m2_l = din("m2_h", (C, C), f16), din("m2_l", (C, C), f16)
    d_wv_h = din("wv_h", (C, C), f16)
    d_wo_h = din("wo_h", (C, C), f16)
    d_gamma, d_beta = din("gamma", (C,)), din("beta", (C,))
    d_bq = din("bq", (1, 2, C), f16)   # pre-scaled by -0.5*C, [hi, lo]
    d_bk = din("bk", (1, 2, C), f16)   # [hi, lo]
    d_bv = din("bv", (1, C), f16)
    d_bo = din("bo", (C,))
    d_gmat = din("gmat", (128, 128))  # block-diag 1/GS group-averaging matrix
    d_out = nc.dram_tensor("out", (C, HW), f32, kind="ExternalOutput").ap()

    with tile.TileContext(nc) as tc, ExitStack() as ctx:
        main = ctx.enter_context(tc.tile_pool(name="main", bufs=1))
        psA = ctx.enter_context(tc.tile_pool(name="psA", bufs=3, space="PSUM"))
        psB = ctx.enter_context(tc.tile_pool(name="psB", bufs=2, space="PSUM"))

        # ---------------- persistent tiles ----------------
        wv_h = main.tile([128, 2, C], f16, name="wv_h")
        wo_h = main.tile([128, 2, C], f16, name="wo_h")
        # wo_h first and via SWDGE: it feeds the very first PE matmuls and
        # must not queue behind the x-chunk loads on the HWDGE queues
        nc.gpsimd.dma_start(wo_h, d_wo_h.rearrange("(kc kl) m -> kl kc m", kl=128))
        if qk_bias:
            wq_h = main.tile([128, 2, C], f16, name="wq_h")
            wq_l = main.tile([128, 2, C], f16, name="wq_l")
            wk_h = main.tile([128, 2, C], f16, name="wk_h")
            wk_l = main.tile([128, 2, C], f16, name="wk_l")
            wloads = [(wk_h, d_wk_h), (wk_l, d_wk_l), (wq_h, d_wq_h),
                      (wq_l, d_wq_l), (wv_h, d_wv_h)]
        else:
            m2_h = main.tile([128, 2, C], f16, name="m2_h")
            m2_l = main.tile([128, 2, C], f16, name="m2_l")
            wloads = [(m2_h, d_m2_h), (m2_l, d_m2_l), (wv_h, d_wv_h)]
        for t, d in wloads:
            nc.sync.dma_start(t, d.rearrange("(kc kl) m -> kl kc m", kl=128))

        # score-matmul operand pairs: (q,k) hi/lo, or (u, hn) hi/lo
        qh = main.tile([128, 2, HW], f16, name="qh")
        ql = None if fp8_cross else main.tile([128, 2, HW], f16, name="ql")
        ul8 = hh8 = None
        if fp8_cross:
            ul8 = main.tile([128, 2, HW], f8e4, name="ul8")
            hh8 = main.tile([128, 2, HW], f8e4, name="hh8")
        if qk_bias:
            kh = main.tile([128, 2, HW], f16, name="kh")
            kl = main.tile([128, 2, HW], f16, name="kl")
            hn_pool = None
        else:
            hn_pool = main   # hn hi/lo persists as the scores rhs
        vT = main.tile([128, NT, C], bf16, name="vT")

        gmat = main.tile([128, 128], f32, name="gmat")
        nc.sync.dma_start(gmat, d_gmat)
        gamma_sb = main.tile([128, 2], f32, name="gamma_sb")
        beta_sb = main.tile([128, 2], f32, name="beta_sb")
        bo_sb = main.tile([128, 2], f32, name="bo_sb")
        nc.sync.dma_start(gamma_sb, d_gamma.rearrange("(t p) -> p t", p=128))
        nc.sync.dma_start(beta_sb, d_beta.rearrange("(t p) -> p t", p=128))
        nc.sync.dma_start(bo_sb, d_bo.rearrange("(t p) -> p t", p=128))
        bq_row = main.tile([1, 2, C], f16, name="bq_row")
        bk_row = main.tile([1, 2, C], f16, name="bk_row")
        bv_row = main.tile([1, C], f16, name="bv_row")
        nc.sync.dma_start(bq_row, d_bq)
        nc.sync.dma_start(bk_row, d_bk)
        nc.sync.dma_start(bv_row, d_bv)
        ones_row = main.tile([1, 512], f16, name="ones_row")
        nc.vector.memset(ones_row, 1.0)
        eps_t = main.tile([128, 2], f32, name="eps_t")
        nc.vector.memset(eps_t, EPS)

        proj_sb = main.tile([128, 2, HW], f16, name="proj_sb")

        with tc.tile_pool(name="pre", bufs=1) as pre:
            # -------------- load x, GroupNorm stats --------------
            # chunked loads/casts so downstream work starts on chunk 0 early
            x_sb = pre.tile([128, 2, HW], f32, name="x_sb")
            d_xv = d_x.rearrange("(t p) n -> p t n", p=128)
            for c8 in range(8):
                for t in range(2):
                    n0 = c8 * 512
                    # first pair via the ACT HWDGE queues: they gate the first
                    # xh cast and all PE work, so skip the SP queue pile-up
                    eng = nc.scalar if c8 == 0 else nc.sync
                    eng.dma_start(x_sb[:, t, n0:n0 + 512],
                                  d_xv[:, t, n0:n0 + 512])

            xh = pre.tile([128, 2, HW], f16, name="xh")
            for c8 in range(8):
                for t in range(2):
                    n0 = c8 * 512
                    nc.scalar.activation(xh[:, t, n0:n0 + 512],
                                         x_sb[:, t, n0:n0 + 512],
                                         mybir.ActivationFunctionType.Copy)

            stats = pre.tile([128, 2, 8, 6], f32, name="stats")
            for t in range(2):
                xv = x_sb[:, t, :].rearrange("p (s n) -> p s n", n=512)
                for s in range(8):
                    nc.vector.bn_stats(stats[:, t, s, :], xv[:, s, :])
            # per-tile stats pipeline: tile 0's scale/bias is ready before
            # tile 1's stats finish, unblocking kc=0 projection matmuls early
            mv = pre.tile([128, 2, 2], f32, name="mv")
            a_sc = pre.tile([128, 2], f32, name="a_sc")
            b_sc = pre.tile([128, 2], f32, name="b_sc")
            stat2 = pre.tile([128, 2, 2], f32, name="stat2")
            gstat = pre.tile([128, 2, 2], f32, name="gstat")
            gvar = pre.tile([128, 2], f32, name="gvar")
            seps = pre.tile([128, 2], f32, name="seps")
            rstd = pre.tile([128, 2], f32, name="rstd")
            tmp = pre.tile([128, 2], f32, name="tmp")
            for t in range(2):
                nc.vector.bn_aggr(mv[:, t, :], stats[:, t, :, :])
                # stat2[:, t] = [mean_t, m2_t]
                nc.vector.tensor_tensor(out=stat2[:, t, 1:2], in0=mv[:, t, 0:1],
                                        in1=mv[:, t, 0:1], op=mybir.AluOpType.mult)
                nc.vector.tensor_tensor(out=stat2[:, t, 1:2], in0=stat2[:, t, 1:2],
                                        in1=mv[:, t, 1:2], op=mybir.AluOpType.add)
                nc.gpsimd.tensor_copy(stat2[:, t, 0:1], mv[:, t, 0:1])
                ps_g = psB.tile([128, 2], f32, name="ps_g", tag="psB")
                nc.tensor.matmul(ps_g, gmat, stat2[:, t, :], start=True, stop=True)
                nc.vector.tensor_copy(gstat[:, t, :], ps_g)
                gm = gstat[:, t, 0:1]
                nc.vector.tensor_tensor(out=gvar[:, t:t + 1], in0=gm, in1=gm,
                                        op=mybir.AluOpType.mult)
                nc.vector.tensor_tensor(out=gvar[:, t:t + 1], in0=gstat[:, t, 1:2],
                                        in1=gvar[:, t:t + 1],
                                        op=mybir.AluOpType.subtract)
                nc.vector.tensor_tensor(out=seps[:, t:t + 1], in0=gvar[:, t:t + 1],
                                        in1=eps_t[:, t:t + 1], op=mybir.AluOpType.add)
                nc.scalar.activation(rstd[:, t:t + 1], seps[:, t:t + 1],
                                     mybir.ActivationFunctionType.Sqrt)
                nc.vector.reciprocal(rstd[:, t:t + 1], rstd[:, t:t + 1])
                for _ in range(2):
                    nc.vector.tensor_tensor(out=tmp[:, t:t + 1], in0=rstd[:, t:t + 1],
                                            in1=rstd[:, t:t + 1], op=mybir.AluOpType.mult)
                    nc.vector.tensor_tensor(out=tmp[:, t:t + 1], in0=tmp[:, t:t + 1],
                                            in1=seps[:, t:t + 1], op=mybir.AluOpType.mult)
                    nc.vector.tensor_scalar(tmp[:, t:t + 1], tmp[:, t:t + 1], -0.5, 1.5,
                                            op0=mybir.AluOpType.mult,
                                            op1=mybir.AluOpType.add)
                    nc.vector.tensor_tensor(out=rstd[:, t:t + 1], in0=rstd[:, t:t + 1],
                                            in1=tmp[:, t:t + 1], op=mybir.AluOpType.mult)
                nc.vector.tensor_tensor(out=a_sc[:, t:t + 1], in0=rstd[:, t:t + 1],
                                        in1=gamma_sb[:, t:t + 1], op=mybir.AluOpType.mult)
                nc.vector.tensor_tensor(out=b_sc[:, t:t + 1], in0=gstat[:, t, 0:1],
                                        in1=a_sc[:, t:t + 1], op=mybir.AluOpType.mult)
                nc.vector.tensor_tensor(out=b_sc[:, t:t + 1], in0=beta_sb[:, t:t + 1],
                                        in1=b_sc[:, t:t + 1], op=mybir.AluOpType.subtract)

            # -------------- out-projection Wo@x + bo -> DRAM scratch ----------
            # (runs on PE while DVE computes hn; uses xh which is pre-norm x)
            for nch in range(8):
                n0 = nch * 512
                for mc in range(2):
                    ps_p = psA.tile([128, 1024], f32, name="ps_p", tag="psA")
                    for kc in range(2):
                        nc.tensor.matmul(ps_p[:, 0:512],
                                         wo_h[:, kc, mc * 128:(mc + 1) * 128],
                                         xh[:, kc, n0:n0 + 512],
                                         start=(kc == 0), stop=(kc == 1))
                    nc.vector.tensor_scalar(proj_sb[:, mc, n0:n0 + 512],
                                            ps_p[:, 0:512],
                                            bo_sb[:, mc:mc + 1], None,
                                            op0=mybir.AluOpType.add)

            # -------------- hn (in-place) and fp16 hi/lo split --------------
            # chunked so the q/k projections can start on early chunks
            hp = hn_pool if hn_pool is not None else pre
            hnh = hp.tile([128, 2, HW], f16, name="hnh")
            hnl = hp.tile([128, 2, HW], f16, name="hnl")
            for t in range(2):
                for c4 in range(4):
                    n0 = c4 * 1024
                    sl = (slice(None), t, slice(n0, n0 + 1024))
                    nc.vector.tensor_scalar(x_sb[sl], x_sb[sl],
                                            a_sc[:, t:t + 1], b_sc[:, t:t + 1],
                                            op0=mybir.AluOpType.mult,
                                            op1=mybir.AluOpType.add)
                    nc.vector.tensor_copy(hnh[sl], x_sb[sl])
                    nc.vector.tensor_tensor(out=hnl[sl], in0=x_sb[sl], in1=hnh[sl],
                                            op=mybir.AluOpType.subtract)

            # -------------- q,k projections (fp16-split, exact-grade) --------
            if fp8_cross and ql is None:
                ql = pre.tile([128, 2, HW], f16, name="ql")
            if qk_bias:
                projs = [(wk_h, wk_l, bk_row, kh, kl),
                         (wq_h, wq_l, bq_row, qh, ql)]
            else:
                projs = [(m2_h, m2_l, None, qh, ql)]   # u = M2^T hn -> qh/ql
            for (w_h, w_l, b_row, out_h, out_l) in projs:
                for mc in range(2):
                    for nh in range(4):
                        n0 = nh * 1024
                        ps_q = psA.tile([128, 1024], f32, name="ps_q", tag="psA")
                        pieces = []
                        for kc in range(2):
                            pieces += [
                                (w_h[:, kc, mc * 128:(mc + 1) * 128], hnh[:, kc]),
                                (w_l[:, kc, mc * 128:(mc + 1) * 128], hnh[:, kc]),
                                (w_h[:, kc, mc * 128:(mc + 1) * 128], hnl[:, kc]),
                            ]
                        for idx, (lhs, rhsrow) in enumerate(pieces):
                            for ns in range(2):
                                j0 = n0 + ns * 512
                                nc.tensor.matmul(ps_q[:, ns * 512:(ns + 1) * 512],
                                                 lhs, rhsrow[:, j0:j0 + 512],
                                                 start=(idx == 0),
                                                 stop=(not qk_bias and idx == len(pieces) - 1))
                        if qk_bias:
                            for hl in range(2):
                                for ns in range(2):
                                    nc.tensor.matmul(ps_q[:, ns * 512:(ns + 1) * 512],
                                                     b_row[:, hl, mc * 128:(mc + 1) * 128],
                                                     ones_row, start=False,
                                                     stop=(hl == 1))
                        nc.scalar.activation(out_h[:, mc, n0:n0 + 1024], ps_q,
                                             mybir.ActivationFunctionType.Copy)
                        nc.vector.tensor_tensor(out=out_l[:, mc, n0:n0 + 1024],
                                                in0=ps_q,
                                                in1=out_h[:, mc, n0:n0 + 1024],
                                                op=mybir.AluOpType.subtract)

            if fp8_cross:
                # product-preserving scales keep the DR result directly
                # accumulable with the fp16 terms in PSUM (no rescale)
                nc.scalar.activation(ul8, ql, mybir.ActivationFunctionType.Copy,
                                     scale=8.0)
                nc.scalar.activation(hh8, hnh, mybir.ActivationFunctionType.Copy,
                                     scale=0.125)

            # -------------- v projection, transposed: vT[i, c] --------------
            for it in range(NT):
                i0 = it * 128
                ps_v = psB.tile([128, C], f32, name="ps_v", tag="psB")
                for kc in range(2):
                    nc.tensor.matmul(ps_v, hnh[:, kc, i0:i0 + 128],
                                     wv_h[:, kc, :], start=(kc == 0),
                                     stop=(not v_bias and kc == 1))
                if v_bias:
                    nc.tensor.matmul(ps_v, ones_row[:, 0:128], bv_row,
                                     start=False, stop=True)
                nc.scalar.activation(vT[:, it, :], ps_v,
                                     mybir.ActivationFunctionType.Copy)

        # ---------------- main attention loop ----------------
        # i-blocks of 512 columns (4 i-tiles) for the AV matmul: N=512 per
        # matmul = 1 psum bank, and the vT stationary operand is loaded once
        # per (jc, mc) per block.
        with tc.tile_pool(name="loop", bufs=2) as lp:
            eT = None
            for t in range(NT):
                i0 = t * 128
                blk, il = t // 4, t % 4
                if il == 0:
                    eT = lp.tile([128, NT, 512], bf16, name="eT", tag="eT", bufs=1)

                # scores for i-tile t: [128, 4096] via 4 psum tiles of 1024;
                # lhsT-major over half-rows: each of the 6 stationary pieces
                # loads once and streams 4 x 512 columns (2 psum tiles).
                sc_sb = lp.tile([128, HW], f32, name="sc_sb", tag="sc")
                rh, rl = (kh, kl) if qk_bias else (hnh, hnl)
                pieces = []
                for kc in range(2):
                    pieces.append((qh[:, kc, i0:i0 + 128], rh[:, kc]))
                    if not fp8_cross:
                        pieces.append((ql[:, kc, i0:i0 + 128], rh[:, kc]))
                    pieces.append((qh[:, kc, i0:i0 + 128], rl[:, kc]))
                for jc in range(4):
                    ps_s = psA.tile([128, 1024], f32, name="ps_s", tag="psA")
                    for idx, (lhs, rhsrow) in enumerate(pieces):
                        for ns in range(2):
                            j0 = jc * 1024 + ns * 512
                            nc.tensor.matmul(ps_s[:, ns * 512:(ns + 1) * 512],
                                             lhs, rhsrow[:, j0:j0 + 512],
                                             start=(idx == 0),
                                             stop=(not fp8_cross and idx == len(pieces) - 1))
                    if fp8_cross:
                        # ul.hh cross-term as one fp8 DoubleRow matmul (K=256
                        # packed via the [Ki,2,dim] interleave; HW-probe-validated)
                        for ns in range(2):
                            j0 = jc * 1024 + ns * 512
                            nc.tensor.matmul(ps_s[:, ns * 512:(ns + 1) * 512],
                                             ul8[:, :, i0:i0 + 128],
                                             hh8[:, :, j0:j0 + 512],
                                             start=False, stop=True,
                                             perf_mode=mybir.MatmulPerfMode.DoubleRow)
                    if fp8_cross and jc % 2 == 1:
                        # fp8 path: ScalarE is co-critical; split copies DVE/ACT
                        nc.vector.tensor_copy(sc_sb[:, jc * 1024:(jc + 1) * 1024],
                                              ps_s)
                    else:
                        nc.scalar.activation(sc_sb[:, jc * 1024:(jc + 1) * 1024],
                                             ps_s, mybir.ActivationFunctionType.Copy)

                e_t = lp.tile([128, HW], bf16, name="e_t", tag="e_t", bufs=3)
                nmax = lp.tile([128, 1], f32, name="nmax", tag="nmax")
                rden = lp.tile([128, 1], f32, name="rden", tag="rden")
                if t < NT - 1:
                    den = lp.tile([128, 1], f32, name="den", tag="den")
                    nc.vector.tensor_reduce(nmax, sc_sb, axis=mybir.AxisListType.X,
                                            op=mybir.AluOpType.max, negate=True)
                    nc.scalar.activation(e_t, sc_sb, mybir.ActivationFunctionType.Exp,
                                         bias=nmax, scale=1.0, accum_out=den)
                    nc.vector.reciprocal(rden, den)
                    nc.vector.tensor_scalar(e_t, e_t, rden, None,
                                            op0=mybir.AluOpType.mult)
                    for tc8 in range(8):
                        nc.sync.dma_start_transpose(
                            eT[:, tc8 * 4:(tc8 + 1) * 4, il * 128:(il + 1) * 128],
                            e_t[:, tc8 * 512:(tc8 + 1) * 512])
                else:
                    # final tile: chunk max/exp/scale/transpose to shorten the
                    # kernel-tail dependency chain
                    cmax = lp.tile([128, 4], f32, name="cmax", tag="cmax")
                    for jc in range(4):
                        nc.vector.tensor_reduce(cmax[:, jc:jc + 1],
                                                sc_sb[:, jc * 1024:(jc + 1) * 1024],
                                                axis=mybir.AxisListType.X,
                                                op=mybir.AluOpType.max)
                    nc.vector.tensor_reduce(nmax, cmax, axis=mybir.AxisListType.X,
                                            op=mybir.AluOpType.max, negate=True)
                    dh = lp.tile([128, 2], f32, name="dh", tag="dh")
                    for hfl in range(2):
                        nc.scalar.activation(e_t[:, hfl * 2048:(hfl + 1) * 2048],
                                             sc_sb[:, hfl * 2048:(hfl + 1) * 2048],
                                             mybir.ActivationFunctionType.Exp,
                                             bias=nmax, scale=1.0,
                                             accum_out=dh[:, hfl:hfl + 1])
                    den2 = lp.tile([128, 1], f32, name="den2", tag="den2")
                    nc.vector.tensor_tensor(out=den2, in0=dh[:, 0:1], in1=dh[:, 1:2],
                                            op=mybir.AluOpType.add)
                    nc.vector.reciprocal(rden, den2)
                    for hfl in range(2):
                        nc.vector.tensor_scalar(e_t[:, hfl * 2048:(hfl + 1) * 2048],
                                                e_t[:, hfl * 2048:(hfl + 1) * 2048],
                                                rden, None, op0=mybir.AluOpType.mult)
                        for tc8 in range(4):
                            c = hfl * 4 + tc8
                            nc.sync.dma_start_transpose(
                                eT[:, c * 4:(c + 1) * 4, il * 128:(il + 1) * 128],
                                e_t[:, c * 512:(c + 1) * 512])

                last_blk = (blk == NT // 4 - 1)
                if il == 2 and last_blk:
                    # partial AV for the final block over its first 3 column
                    # stripes (N=384) so the kernel tail only owes the last
                    # 128-column stripe after the final softmax chain.
                    ps_h = [psB.tile([128, 512], f32, name=f"ps_h{m}", tag="psB")
                            for m in range(2)]
                    for jc in range(NT):
                        for mc in range(2):
                            nc.tensor.matmul(ps_h[mc][:, 0:384],
                                             vT[:, jc, mc * 128:(mc + 1) * 128],
                                             eT[:, jc, 0:384],
                                             start=(jc == 0), stop=(jc == NT - 1))
                if il == 3:
                    # AV for block blk: h[c, i] = sum_j vT[j,c] * eT[j, i]
                    if not last_blk:
                        ps_h = [psB.tile([128, 512], f32, name=f"ps_h{m}", tag="psB")
                                for m in range(2)]
                    for jc in range(NT):
                        for mc in range(2):
                            sl = slice(384, 512) if last_blk else slice(0, 512)
                            nc.tensor.matmul(ps_h[mc][:, sl],
                                             vT[:, jc, mc * 128:(mc + 1) * 128],
                                             eT[:, jc, sl],
                                             start=(jc == 0), stop=(jc == NT - 1))
                    for mc in range(2):
                        o_sb = lp.tile([128, 512], f32, name="o_sb", tag="o_sb")
                        nc.vector.tensor_tensor(out=o_sb, in0=ps_h[mc],
                                                in1=proj_sb[:, mc, blk * 512:(blk + 1) * 512],
                                                op=mybir.AluOpType.add)
                        nc.sync.dma_start(
                            d_out[mc * 128:(mc + 1) * 128, blk * 512:(blk + 1) * 512],
                            o_sb)

    _dedup_ldweights(nc)
    nc.compile()
    return nc


def _build_program_fast(v_bias, fp8_terms=2):
    """Optimized zero-qk-bias path.

    Scores s = u^T hn with u = M2^T hn (M2 = (Wq*-0.5C)^T Wk).  fp16 hi/lo
    split of u and hn gives s = uh^T hh (fp16, exact-grade) plus two small
    cross terms ul^T hh and uh^T hl, each run as ONE fp8e4 DoubleRow matmul
    with K=256 packed via the [Ki,2,dim] interleave at 0.5 cyc/row (product-
    preserving scale pairs keep PSUM accumulation direct).  Softmax is fully
    chunked: per 1024-col chunk, DVE takes -max from PSUM, ACT exps straight
    out of PSUM (bias=-chunk-max, accum_out=chunk denom) into bf16 SBUF, and
    a per-chunk fixup scale e^{m_jc-m}/den folds the global max AND the
    normalization into one small multiply -- no fp32 score copy to SBUF at
    all.
    """
    import os as _os
    # InstTensorTensorReduce crashes real TRN2 (NRT_EXEC_UNIT_UNRECOVERABLE,
    # HW-bisected 2026-08-07) even though CoreSim/TimelineSim accept it --
    # always use the separate tensor_tensor + tensor_reduce pair instead.
    no_ttr = True
    exp_sbuf = bool(_os.environ.get("ATTN_EXP_SBUF"))
    tr512 = bool(_os.environ.get("ATTN_TR512"))
    no_negscale = bool(_os.environ.get("ATTN_NO_NEGSCALE"))

    nc = bacc.Bacc("TRN2", target_bir_lowering=False, debug=False)

    def din(name, shape, dt=f32):
        return nc.dram_tensor(name, shape, dt, kind="ExternalInput").ap()

    d_x = din("x", (C, HW))
    d_m2_h, d_m2_l = din("m2_h", (C, C), f16), din("m2_l", (C, C), f16)
    d_wv_h = din("wv_h", (C, C), f16)
    d_wo_h = din("wo_h", (C, C), f16)
    d_gamma, d_beta = din("gamma", (C,)), din("beta", (C,))
    d_bv = din("bv", (1, C), f16)
    d_bo16 = din("bo16", (1, C), f16)
    d_gmat = din("gmat", (128, 128))
    d_out = nc.dram_tensor("out", (C, HW), f16, kind="ExternalOutput").ap()

    DR = mybir.MatmulPerfMode.DoubleRow
    f32r = mybir.dt.float32r

    with tile.TileContext(nc) as tc, ExitStack() as ctx:
        main = ctx.enter_context(tc.tile_pool(name="main", bufs=1))
        psA = ctx.enter_context(tc.tile_pool(name="psA", bufs=3, space="PSUM"))
        psB = ctx.enter_context(tc.tile_pool(name="psB", bufs=2, space="PSUM"))

        # ---------------- persistent tiles ----------------
        wv_h = main.tile([128, 2, C], f16, name="wv_h")
        m2_h = main.tile([128, 2, C], f16, name="m2_h")
        m2_l = main.tile([128, 2, C], f16, name="m2_l")
        for tl_, d in [(m2_h, d_m2_h), (m2_l, d_m2_l), (wv_h, d_wv_h)]:
            nc.sync.dma_start(tl_, d.rearrange("(kc kl) m -> kl kc m", kl=128))

        qh = main.tile([128, 2, HW], f16, name="qh")
        hnh = main.tile([128, 2, HW], f16, name="hnh")
        ul8 = main.tile([128, 2, HW], f8e4, name="ul8")
        hh8 = main.tile([128, 2, HW], f8e4, name="hh8")
        if fp8_terms == 2:
            uh8 = main.tile([128, 2, HW], f8e4, name="uh8")
            hl8 = main.tile([128, 2, HW], f8e4, name="hl8")
            # fp8 copies of M2 for the u-projection cross terms, scale-paired
            # with the existing hh8 (hn/8) and hl8 (hn_lo*256)
            m2l8 = main.tile([128, 2, C], f8e4, name="m2l8")
            m2h8 = main.tile([128, 2, C], f8e4, name="m2h8")
        vT = main.tile([128, NT, C], bf16, name="vT")
        # xh16: f16 copy of x for the out-projection Wo@x, which runs folded
        # into each block's AV PSUM accumulation (no separate proj pass, no
        # proj_sb; f16 is ample for this post-softmax path).  NOTE fp32r
        # matmuls straight off the DMA-landed fp32 x are rejected by the BIR
        # verifier (operands must be rounded to fp32r by a compute engine),
        # so the f16 cast path is the cheap legal option.
        xh16 = main.tile([128, 2, HW], f16, name="xh16")
        wo_h = main.tile([128, 2, C], f16, name="wo_h")
        nc.gpsimd.dma_start(wo_h, d_wo_h.rearrange("(kc kl) m -> kl kc m", kl=128))

        gamma_sb = main.tile([128, 2], f32, name="gamma_sb")
        beta_sb = main.tile([128, 2], f32, name="beta_sb")
        nc.sync.dma_start(gamma_sb, d_gamma.rearrange("(t p) -> p t", p=128))
        nc.sync.dma_start(beta_sb, d_beta.rearrange("(t p) -> p t", p=128))
        bo_row = main.tile([1, C], f16, name="bo_row")
        nc.sync.dma_start(bo_row, d_bo16)
        bv_row = main.tile([1, C], f16, name="bv_row")
        nc.sync.dma_start(bv_row, d_bv)
        ones_row = main.tile([1, 512], f16, name="ones_row")
        nc.vector.memset(ones_row, 1.0)
        eps_t = main.tile([128, 2], f32, name="eps_t")
        nc.vector.memset(eps_t, EPS)

        with tc.tile_pool(name="pre", bufs=1) as pre:
            gmat = pre.tile([128, 128], f32, name="gmat")
            nc.sync.dma_start(gmat, d_gmat)
            # p-state warmup: the PE would otherwise idle ~20us during the x
            # load and then pay the 2-3.7x cold-clock penalty on the first
            # real matmuls.  Chew on the already-landed m2 weights back to
            # back so the ramp (3us continuous -> full clock) completes
            # before the projections start.
            n_warm = int(_os.environ.get("ATTN_WARM", "0"))
            for w in range(n_warm):
                ps_w = psA.tile([128, 1024], f32, name="ps_w", tag="psA")
                nc.tensor.matmul(ps_w[:, 0:512], m2_h[:, 0, 0:128],
                                 m2_h.rearrange("p a b -> p (a b)")[:, 0:512],
                                 start=True, stop=True)
            if fp8_terms == 2:
                nc.scalar.activation(m2l8, m2_l,
                                     mybir.ActivationFunctionType.Copy,
                                     scale=8.0)
                nc.scalar.activation(m2h8, m2_h,
                                     mybir.ActivationFunctionType.Copy,
                                     scale=1.0 / 256.0)

            # -------------- load x, GroupNorm stats --------------
            x_sb = pre.tile([128, 2, HW], f32, name="x_sb")
            d_xv = d_x.rearrange("(t p) n -> p t n", p=128)
            for c8 in range(8):
                for t in range(2):
                    n0 = c8 * 512
                    eng = nc.scalar if c8 == 0 else nc.sync
                    eng.dma_start(x_sb[:, t, n0:n0 + 512],
                                  d_xv[:, t, n0:n0 + 512])

            # f16 copy of x chunks as the x DMAs land (for the fused out-proj)
            for c8 in range(8):
                for t in range(2):
                    n0 = c8 * 512
                    nc.scalar.activation(xh16[:, t, n0:n0 + 512],
                                         x_sb[:, t, n0:n0 + 512],
                                         mybir.ActivationFunctionType.Copy)

            stats = pre.tile([128, 2, 8, 6], f32, name="stats")
            stats_src = xh16 if _os.environ.get("ATTN_STATS16") else x_sb
            for t in range(2):
                xv = stats_src[:, t, :].rearrange("p (s n) -> p s n", n=512)
                for s in range(8):
                    nc.vector.bn_stats(stats[:, t, s, :], xv[:, s, :])
            mv = pre.tile([128, 2, 2], f32, name="mv")
            a_sc = pre.tile([128, 2], f32, name="a_sc")
            b_sc = pre.tile([128, 2], f32, name="b_sc")
            stat2 = pre.tile([128, 2, 2], f32, name="stat2")
            gstat = pre.tile([128, 2, 2], f32, name="gstat")
            gvar = pre.tile([128, 2], f32, name="gvar")
            seps = pre.tile([128, 2], f32, name="seps")
            rstd = pre.tile([128, 2], f32, name="rstd")
            tmp = pre.tile([128, 2], f32, name="tmp")
            for t in range(2):
                nc.vector.bn_aggr(mv[:, t, :], stats[:, t, :, :])
                nc.vector.tensor_tensor(out=stat2[:, t, 1:2], in0=mv[:, t, 0:1],
                                        in1=mv[:, t, 0:1], op=mybir.AluOpType.mult)
                nc.vector.tensor_tensor(out=stat2[:, t, 1:2], in0=stat2[:, t, 1:2],
                                        in1=mv[:, t, 1:2], op=mybir.AluOpType.add)
                # mean copy on DVE: a Pool round trip here puts Q7 launch
                # + two sem hops into the stats critical chain
                nc.vector.tensor_copy(stat2[:, t, 0:1], mv[:, t, 0:1])
                ps_g = psB.tile([128, 2], f32, name="ps_g", tag="psB")
                nc.tensor.matmul(ps_g, gmat, stat2[:, t, :], start=True, stop=True)
                nc.vector.tensor_copy(gstat[:, t, :], ps_g)
                gm = gstat[:, t, 0:1]
                nc.vector.tensor_tensor(out=gvar[:, t:t + 1], in0=gm, in1=gm,
                                        op=mybir.AluOpType.mult)
                nc.vector.tensor_tensor(out=gvar[:, t:t + 1], in0=gstat[:, t, 1:2],
                                        in1=gvar[:, t:t + 1],
                                        op=mybir.AluOpType.subtract)
                nc.vector.tensor_tensor(out=seps[:, t:t + 1], in0=gvar[:, t:t + 1],
                                        in1=eps_t[:, t:t + 1], op=mybir.AluOpType.add)
                nc.scalar.activation(rstd[:, t:t + 1], seps[:, t:t + 1],
                                     mybir.ActivationFunctionType.Sqrt)
                nc.vector.reciprocal(rstd[:, t:t + 1], rstd[:, t:t + 1])
                for _ in range(int(_os.environ.get("ATTN_NR", "0"))):
                    nc.vector.tensor_tensor(out=tmp[:, t:t + 1], in0=rstd[:, t:t + 1],
                                            in1=rstd[:, t:t + 1], op=mybir.AluOpType.mult)
                    nc.vector.tensor_tensor(out=tmp[:, t:t + 1], in0=tmp[:, t:t + 1],
                                            in1=seps[:, t:t + 1], op=mybir.AluOpType.mult)
                    nc.vector.tensor_scalar(tmp[:, t:t + 1], tmp[:, t:t + 1], -0.5, 1.5,
                                            op0=mybir.AluOpType.mult,
                                            op1=mybir.AluOpType.add)
                    nc.vector.tensor_tensor(out=rstd[:, t:t + 1], in0=rstd[:, t:t + 1],
                                            in1=tmp[:, t:t + 1], op=mybir.AluOpType.mult)
                nc.vector.tensor_tensor(out=a_sc[:, t:t + 1], in0=rstd[:, t:t + 1],
                                        in1=gamma_sb[:, t:t + 1], op=mybir.AluOpType.mult)
                nc.vector.tensor_tensor(out=b_sc[:, t:t + 1], in0=gstat[:, t, 0:1],
                                        in1=a_sc[:, t:t + 1], op=mybir.AluOpType.mult)
                nc.vector.tensor_tensor(out=b_sc[:, t:t + 1], in0=beta_sb[:, t:t + 1],
                                        in1=b_sc[:, t:t + 1], op=mybir.AluOpType.subtract)

            # -------------- hn (scratch) and fp16 hi/lo split --------------
            # hh8 on ACT; hl8 on the otherwise-idle Pool engine.  fp8 copies
            # are chunk-interleaved so they pipeline behind the DVE hn chunks
            hnl = (pre if fp8_terms == 2 else main).tile(
                [128, 2, HW], f16, name="hnl")
            for c4 in range(4):
                for t in range(2):
                    n0 = c4 * 1024
                    sl = (slice(None), t, slice(n0, n0 + 1024))
                    scr = pre.tile([128, 1024], f32, name="hnscr",
                                   tag="hnscr", bufs=2)
                    nc.vector.tensor_scalar(scr, x_sb[sl],
                                            a_sc[:, t:t + 1], b_sc[:, t:t + 1],
                                            op0=mybir.AluOpType.mult,
                                            op1=mybir.AluOpType.add)
                    # hnh extract stays on DVE: it sits on the serial hn
                    # spine (hnl needs it) and a cross-engine hop here costs
                    # more latency than DVE throughput
                    nc.vector.tensor_copy(hnh[sl], scr)
                    nc.vector.tensor_tensor(out=hnl[sl], in0=scr, in1=hnh[sl],
                                            op=mybir.AluOpType.subtract)
                    nc.scalar.activation(hh8[sl], hnh[sl],
                                         mybir.ActivationFunctionType.Copy,
                                         scale=0.125)
                    if fp8_terms == 2:
                        nc.gpsimd.tensor_scalar(hl8[sl], hnl[sl], 256.0, None,
                                                op0=mybir.AluOpType.mult)

            # ---- u projection (fp16-split, exact-grade) + v projection ----
            # interleaved per 1024-column chunk so the DVE queue (ql extract,
            # vT copies) drains alongside the PE stream and the first score
            # maxes aren't stuck behind 32 queued vT copies
            ql = pre.tile([128, 2, HW], f16, name="ql")
            for nh in range(4):
                n0 = nh * 1024
                for mc in range(2):
                    ps_q = psA.tile([128, 1024], f32, name="ps_q", tag="psA")
                    if fp8_terms == 2:
                        # hi-hi fp16 + both cross terms as fp8 DoubleRow
                        # (K=256 packed), reusing the scores' hh8/hl8
                        pieces = [
                            (m2_h[:, 0, mc * 128:(mc + 1) * 128], hnh[:, 0], None),
                            (m2_h[:, 1, mc * 128:(mc + 1) * 128], hnh[:, 1], None),
                            (m2l8[:, :, mc * 128:(mc + 1) * 128], hh8, DR),
                            (m2h8[:, :, mc * 128:(mc + 1) * 128], hl8, DR),
                        ]
                    else:
                        pieces = []
                        for kc in range(2):
                            pieces += [
                                (m2_h[:, kc, mc * 128:(mc + 1) * 128], hnh[:, kc], None),
                                (m2_l[:, kc, mc * 128:(mc + 1) * 128], hnh[:, kc], None),
                                (m2_h[:, kc, mc * 128:(mc + 1) * 128], hnl[:, kc], None),
                            ]
                    for idx, (lhs, rhsrow, pm) in enumerate(pieces):
                        for ns in range(2):
                            j0 = n0 + ns * 512
                            psl = ps_q[:, ns * 512:(ns + 1) * 512]
                            st, sp = (idx == 0), (idx == len(pieces) - 1)
                            if pm is None:
                                nc.tensor.matmul(psl, lhs,
                                                 rhsrow[:, j0:j0 + 512],
                                                 start=st, stop=sp)
                            else:
                                nc.tensor.matmul(psl, lhs,
                                                 rhsrow[:, :, j0:j0 + 512],
                                                 start=st, stop=sp,
                                                 perf_mode=pm)
                    nc.scalar.activation(qh[:, mc, n0:n0 + 1024], ps_q,
                                         mybir.ActivationFunctionType.Copy)
                    nc.vector.tensor_tensor(out=ql[:, mc, n0:n0 + 1024],
                                            in0=ps_q,
                                            in1=qh[:, mc, n0:n0 + 1024],
                                            op=mybir.AluOpType.subtract)
                    # chunked fp8 copies pipeline behind the projection
                    nc.scalar.activation(ul8[:, mc, n0:n0 + 1024],
                                         ql[:, mc, n0:n0 + 1024],
                                         mybir.ActivationFunctionType.Copy,
                                         scale=8.0)
                    if fp8_terms == 2:
                        nc.gpsimd.tensor_scalar(uh8[:, mc, n0:n0 + 1024],
                                                qh[:, mc, n0:n0 + 1024],
                                                1.0 / 256.0, None,
                                                op0=mybir.AluOpType.mult)
                pass  # v-projection is emitted inside the main loop (after
                # scores tile 0): vT is first read by AV(0) at tile 4, so
                # deferring it starts the score loop one pipeline stage sooner

        # ---------------- main attention loop ----------------
        # Two levels of software pipelining keep every engine streaming:
        #  * tile level: the softmax tail of tile t-1 (global-max fixup chain,
        #    which round-trips DVE->ACT->DVE, the fixup scales, and the eT
        #    transposes) is emitted in the middle of tile t's chunk stream, so
        #    DVE's in-order queue overlaps the ACT round trip with tile t's
        #    chunk maxes instead of head-of-line blocking on it.
        #  * block level: AV for block b is emitted AFTER the score matmuls of
        #    tile 4b+4 so the PE has score work while block b's tail drains
        #    (eT is double-buffered to allow it).
        with tc.tile_pool(name="loop", bufs=2) as lp:
            eT_blk = {}
            state = {}
            ps_h = None

            tail_a = {}

            def emit_tail_a(t):
                """Tail part A for tile t: everything up to (and including)
                the DVE->ACT round trip for e^{m_jc - m}.  Emitted separately
                from part B so the next chunk's max can run on DVE while the
                ACT hop is in flight."""
                cmax, denj, e_t = state.pop(t)
                nm = lp.tile([128, 1], f32, name="nm", tag="nm")
                fsc = lp.tile([128, 4], f32, name="fsc", tag="fsc")
                nc.vector.tensor_reduce(nm, cmax, axis=mybir.AxisListType.X,
                                        op=mybir.AluOpType.min)  # nm = -m
                nc.vector.tensor_scalar(fsc, cmax, nm, None,
                                        op0=mybir.AluOpType.subtract)  # m - m_jc
                nc.scalar.activation(fsc, fsc,
                                     mybir.ActivationFunctionType.Exp,
                                     scale=-1.0)  # e^{m_jc - m}
                tail_a[t] = (denj, e_t, fsc)

            def emit_tail_b(t):
                """Tail part B: denominator combine, fixup scales, transposes."""
                denj, e_t, fsc = tail_a.pop(t)
                eT = eT_blk[t // 4]
                il = t % 4
                dsum = lp.tile([128, 4], f32, name="dsum", tag="dsum")
                dden = lp.tile([128, 1], f32, name="dden", tag="dden")
                rden = lp.tile([128, 1], f32, name="rden", tag="rden")
                g = lp.tile([128, 4], f32, name="g", tag="g")
                if no_ttr:
                    nc.vector.tensor_tensor(out=dsum, in0=denj, in1=fsc,
                                            op=mybir.AluOpType.mult)
                    nc.vector.tensor_reduce(dden, dsum,
                                            axis=mybir.AxisListType.X,
                                            op=mybir.AluOpType.add)
                else:
                    nc.vector.tensor_tensor_reduce(out=dsum, in0=denj, in1=fsc,
                                                   scale=1.0, scalar=0.0,
                                                   op0=mybir.AluOpType.mult,
                                                   op1=mybir.AluOpType.add,
                                                   accum_out=dden)
                nc.vector.reciprocal(rden, dden)
                nc.vector.tensor_scalar(g, fsc, rden, None,
                                        op0=mybir.AluOpType.mult)
                for jc in range(4):
                    # first two fixups ride the idle Pool engine (their
                    # transposes have slack); the last two stay on DVE so the
                    # block's eT completes before the delayed AV needs it
                    fix_pool = int(_os.environ.get("ATTN_FIXPOOL", "0"))
                    if jc < fix_pool:
                        nc.gpsimd.tensor_scalar(e_t[:, jc * 1024:(jc + 1) * 1024],
                                                e_t[:, jc * 1024:(jc + 1) * 1024],
                                                g[:, jc:jc + 1], None,
                                                op0=mybir.AluOpType.mult)
                    else:
                        nc.vector.tensor_scalar(e_t[:, jc * 1024:(jc + 1) * 1024],
                                                e_t[:, jc * 1024:(jc + 1) * 1024],
                                                g[:, jc:jc + 1], None,
                                                op0=mybir.AluOpType.mult)
                    if tr512:
                        for h2 in range(2):
                            cc = jc * 2 + h2
                            nc.sync.dma_start_transpose(
                                eT[:, cc * 4:(cc + 1) * 4, il * 128:(il + 1) * 128],
                                e_t[:, cc * 512:(cc + 1) * 512])
                    else:
                        nc.sync.dma_start_transpose(
                            eT[:, jc * 8:(jc + 1) * 8, il * 128:(il + 1) * 128],
                            e_t[:, jc * 1024:(jc + 1) * 1024])

            def emit_tail(t):
                emit_tail_a(t)
                emit_tail_b(t)

            def emit_av_open(b, sl):
                # out-projection (Wo@x + bo) opens each stripe's PSUM group;
                # it depends only on xh16, so at block boundaries it gives the
                # PE dep-free work while DVE/ACT drain the chunk backlog
                g0 = b * 512 + sl.start
                gn = sl.stop - sl.start
                for mc in range(2):
                    for kc in range(2):
                        nc.tensor.matmul(ps_h[mc][:, sl],
                                         wo_h[:, kc, mc * 128:(mc + 1) * 128],
                                         xh16[:, kc, g0:g0 + gn],
                                         start=(kc == 0), stop=False)
                    nc.tensor.matmul(ps_h[mc][:, sl],
                                     bo_row[:, mc * 128:(mc + 1) * 128],
                                     ones_row[:, 0:gn],
                                     start=False, stop=False)

            def emit_av_body(b, sl):
                eTb = eT_blk[b]
                for jc in range(NT):
                    for mc in range(2):
                        nc.tensor.matmul(ps_h[mc][:, sl],
                                         vT[:, jc, mc * 128:(mc + 1) * 128],
                                         eTb[:, jc, sl],
                                         start=False, stop=(jc == NT - 1))

            def emit_av(b, sl):
                emit_av_open(b, sl)
                emit_av_body(b, sl)

            def emit_out(b, sl=slice(0, 512)):
                g0 = b * 512 + sl.start
                gn = sl.stop - sl.start
                for mc in range(2):
                    o_sb = lp.tile([128, 512], f16, name="o_sb", tag="o_sb")
                    nc.vector.tensor_copy(o_sb[:, 0:gn], ps_h[mc][:, sl])
                    nc.sync.dma_start(
                        d_out[mc * 128:(mc + 1) * 128, g0:g0 + gn],
                        o_sb[:, 0:gn])

            for t in range(NT):
                i0 = t * 128
                blk, il = t // 4, t % 4
                if t == int(_os.environ.get("ATTN_VPOS", "3")):
                    # deferred v-projection: runs on PE after scores tile 0,
                    # well before its first reader AV(0) at tile 4
                    for it in range(NT):
                        iv = it * 128
                        ps_v = psB.tile([128, C], f32, name="ps_v", tag="psB")
                        for kc in range(2):
                            nc.tensor.matmul(ps_v, hnh[:, kc, iv:iv + 128],
                                             wv_h[:, kc, :], start=(kc == 0),
                                             stop=(not v_bias and kc == 1))
                        if v_bias:
                            nc.tensor.matmul(ps_v, ones_row[:, 0:128], bv_row,
                                             start=False, stop=True)
                        if it % 2 == 0:
                            nc.vector.tensor_copy(vT[:, it, :], ps_v)
                        else:
                            nc.scalar.activation(vT[:, it, :], ps_v,
                                                 mybir.ActivationFunctionType.Copy)
                if il == 0:
                    eT_blk[blk] = lp.tile([128, NT, 512], bf16, name="eT",
                                          tag="eT", bufs=2)
                    eT_blk.pop(blk - 2, None)

                e_t = lp.tile([128, HW], bf16, name="e_t", tag="e_t", bufs=4)
                cmax = lp.tile([128, 4], f32, name="cmax", tag="cmax")
                denj = lp.tile([128, 4], f32, name="denj", tag="denj")
                state[t] = (cmax, denj, e_t)

                boundary_av = (il == 0 and blk >= 1)

                pieces = [
                    (qh[:, 0, i0:i0 + 128], hnh[:, 0], None),
                    (qh[:, 1, i0:i0 + 128], hnh[:, 1], None),
                ]
                if fp8_terms < 2:
                    pieces += [(qh[:, 0, i0:i0 + 128], hnl[:, 0], None),
                               (qh[:, 1, i0:i0 + 128], hnl[:, 1], None)]
                    # re-order so each qh half loads once (ldweights dedup)
                    pieces = [pieces[0], pieces[2], pieces[1], pieces[3]]
                pieces.append((ul8[:, :, i0:i0 + 128], hh8, DR))
                if fp8_terms == 2:
                    pieces.append((uh8[:, :, i0:i0 + 128], hl8, DR))

                for jc in range(4):
                    ps_s = psA.tile([128, 1024], f32, name="ps_s", tag="psA")
                    for idx, (lhs, rhs, pm) in enumerate(pieces):
                        for ns in range(2):
                            j0 = jc * 1024 + ns * 512
                            psl = ps_s[:, ns * 512:(ns + 1) * 512]
                            st, sp = (idx == 0), (idx == len(pieces) - 1)
                            if pm is None:
                                nc.tensor.matmul(psl, lhs, rhs[:, j0:j0 + 512],
                                                 start=st, stop=sp)
                            else:
                                nc.tensor.matmul(psl, lhs, rhs[:, :, j0:j0 + 512],
                                                 start=st, stop=sp, perf_mode=pm)
                    if exp_sbuf:
                        sc_sb = lp.tile([128, 1024], f32, name="sc_sb",
                                        tag="sc_sb", bufs=3)
                        nc.scalar.activation(sc_sb, ps_s,
                                             mybir.ActivationFunctionType.Copy)
                        src = sc_sb
                    else:
                        src = ps_s
                    nc.vector.tensor_reduce(cmax[:, jc:jc + 1], src,
                                            axis=mybir.AxisListType.X,
                                            op=mybir.AluOpType.max, negate=True)
                    nc.scalar.activation(e_t[:, jc * 1024:(jc + 1) * 1024], src,
                                         mybir.ActivationFunctionType.Exp,
                                         bias=cmax[:, jc:jc + 1], scale=1.0,
                                         accum_out=denj[:, jc:jc + 1])
                    if jc == 1 and t >= 1 and (t - 1) in state:
                        emit_tail(t - 1)

                last_blk = (blk == NT // 4 - 1)
                if boundary_av:
                    ps_h = [psB.tile([128, 512], f32, name=f"ps_h{m}", tag="psB")
                            for m in range(2)]
                    emit_av(blk - 1, slice(0, 512))
                    emit_out(blk - 1)
                if last_blk and il == 2:
                    # final block: AV in stripes, each gated only on tails
                    # that are already emitted, so the PE never waits; each
                    # stripe's output ships as soon as its group stops
                    ps_h = [psB.tile([128, 512], f32, name=f"ps_h{m}", tag="psB")
                            for m in range(2)]
                    emit_av(blk, slice(0, 256))   # tiles 28,29 tails done
                if last_blk and il == 3:
                    emit_av(blk, slice(256, 384))  # tile 30 tail done (jc1)
                    emit_tail(t)
                    emit_av(blk, slice(384, 512))
                    emit_out(blk)

    _dedup_ldweights(nc)
    nc.compile()
    return nc


def _dedup_ldweights(nc):
    """Remove back-to-back InstLdweights that reload the identical stationary
    operand on the PE stream (tile splits every matmul into ldweights+matmult,
    even when consecutive matmuls share weights). Any sync info on a removed
    load is merged into the following kept PE instruction."""
    import concourse.mybir as mybir_m

    for f in nc.m.functions:
        for blk in f.blocks:
            insts = blk.instructions
            last_key = None
            pending_waits = []
            pending_updates = []
            keep = []
            removed = 0
            for inst in insts:
                tn = type(inst).__name__
                eng = str(inst.engine)
                if "PE" not in eng:
                    keep.append(inst)
                    continue
                if tn == "InstLdweights":
                    a = inst.ins[0]
                    key = (getattr(a, "memref", None), getattr(a, "offset", None),
                           str(getattr(a, "ap", None)), str(getattr(a, "dtype", None)))
                    if key == last_key:
                        si = inst.sync_info
                        if si is not None:
                            pending_waits += list(si.on_wait)
                            pending_updates += list(si.on_update)
                        removed += 1
                        continue
                    last_key = key
                elif tn == "InstMatmult":
                    # fp32/fp32r matmuls self-load their weights (no separate
                    # InstLdweights), clobbering the PE array state
                    d = str(getattr(inst.ins[0], "dtype", ""))
                    if "float32" in d:
                        last_key = None
                else:
                    # unknown PE instruction: weights state no longer certain
                    last_key = None
                if (pending_waits or pending_updates):
                    si = inst.sync_info
                    if si is None:
                        inst.sync_info = mybir_m.SyncInfo(
                            on_wait=pending_waits, on_update=pending_updates)
                    else:
                        inst.sync_info = mybir_m.SyncInfo(
                            on_wait=list(si.on_wait) + pending_waits,
                            on_update=list(si.on_update) + pending_updates)
                    pending_waits, pending_updates = [], []
                keep.append(inst)
            if removed:
                while len(blk.instructions):
                    blk.instructions.pop()
                for inst in keep:
                    blk.instructions.append(inst)


def _get_program(qk_bias=True, v_bias=True, fp8_cross=False):
    key = (qk_bias, v_bias, fp8_cross)
    if key not in _PROGRAMS:
        _PROGRAMS[key] = _build_program(qk_bias, v_bias, fp8_cross)
    return _PROGRAMS[key]


def _get_program_fast(v_bias=False, fp8_terms=2):
    import os as _os
    knobs = tuple(bool(_os.environ.get(k)) for k in
                  ("ATTN_NO_TTR", "ATTN_EXP_SBUF", "ATTN_TR512",
                   "ATTN_NO_NEGSCALE"))
    key = ("fast", v_bias, fp8_terms, knobs)
    if key not in _PROGRAMS:
        _PROGRAMS[key] = _build_program_fast(v_bias, fp8_terms)
    return _PROGRAMS[key]


def kernel(x, norm_gamma, norm_beta, Wq, bq, Wk, bk, Wv, bv, Wo, bo):
    x = np.ascontiguousarray(np.asarray(x, np.float32))
    assert x.shape == (B, C, H, W)

    def _bias_hl(b32):
        h = b32.astype(np.float16)
        l = (b32 - h.astype(np.float32)).astype(np.float16)
        return np.stack([h, l]).reshape(1, 2, C)

    def split16(w):
        h = w.astype(np.float16)
        l = (w - h.astype(np.float32)).astype(np.float16)
        return h, l

    scale = -0.5 * C
    wq_t = np.ascontiguousarray((np.asarray(Wq, np.float32) * scale).T)
    wk_t = np.ascontiguousarray(np.asarray(Wk, np.float32).T)
    wv_t = np.ascontiguousarray(np.asarray(Wv, np.float32).T)
    wo_t = np.ascontiguousarray(np.asarray(Wo, np.float32).T)
    wq_h, wq_l = split16(wq_t)
    wk_h, wk_l = split16(wk_t)
    # exact bilinear fold for the zero-bias fast path: s = hn^T M2 hn
    m2 = ((np.asarray(Wq, np.float64) * scale).T @ np.asarray(Wk, np.float64))
    m2_h = m2.astype(np.float16)
    m2_l = (m2 - m2_h.astype(np.float64)).astype(np.float16)
    wv_h = wv_t.astype(np.float16)
    wo_h = wo_t.astype(np.float16)

    gmat = np.zeros((128, 128), np.float32)
    for g in range(128 // GS):
        gmat[g * GS:(g + 1) * GS, g * GS:(g + 1) * GS] = 1.0 / GS

    common = {
        "wq_h": wq_h, "wq_l": wq_l, "wk_h": wk_h, "wk_l": wk_l,
        "wv_h": wv_h, "wo_h": wo_h, "wo32": wo_t,
        "m2_h": np.ascontiguousarray(m2_h), "m2_l": np.ascontiguousarray(m2_l),
        "gamma": np.asarray(norm_gamma, np.float32),
        "beta": np.asarray(norm_beta, np.float32),
        "bq": _bias_hl(np.asarray(bq, np.float32) * scale),
        "bk": _bias_hl(np.asarray(bk, np.float32)),
        "bv": np.asarray(bv, np.float32).astype(np.float16).reshape(1, C),
        "bo": np.asarray(bo, np.float32),
        "bo16": np.asarray(bo, np.float32).astype(np.float16).reshape(1, C),
        "gmat": gmat,
    }
    in_maps = [dict(common, x=x[c].reshape(C, HW)) for c in range(B)]

    qk_bias = bool(np.any(np.asarray(bq)) or np.any(np.asarray(bk)))
    v_bias = bool(np.any(np.asarray(bv)))
    import os as _os
    if qk_bias:
        nc = _get_program(qk_bias, v_bias, False)
    else:
        terms = int(_os.environ.get("ATTN_FP8_TERMS", "2"))
        if terms == 0:
            nc = _get_program(False, v_bias, False)
        else:
            nc = _get_program_fast(v_bias, terms)
    global _LAST_PROGRAM
    _LAST_PROGRAM = nc
    import os
    trace = bool(os.environ.get("ATTN_TRACE"))
    res = run_bass_kernel_spmd(nc, in_maps, core_ids=list(range(B)),
                               trace=trace,
                               tmpdir=os.environ.get("ATTN_TRACE_DIR") or None)
    global _LAST_EXEC_NS
    _LAST_EXEC_NS = res.exec_time_ns
    out = np.stack([res.results[c]["out"].reshape(C, H, W) for c in range(B)])
    return out.astype(np.float32)


_LAST_EXEC_NS = None
_LAST_PROGRAM = None


if __name__ == "__main__":
    rng = np.random.default_rng(0)
    ins = {
        "x": rng.standard_normal((B, C, H, W)).astype(np.float32),
        "norm_gamma": np.ones(C, np.float32),
        "norm_beta": np.zeros(C, np.float32),
        "Wq": (rng.standard_normal((C, C)) / 16).astype(np.float32),
        "bq": np.zeros(C, np.float32),
        "Wk": (rng.standard_normal((C, C)) / 16).astype(np.float32),
        "bk": np.zeros(C, np.float32),
        "Wv": (rng.standard_normal((C, C)) / 16).astype(np.float32),
        "bv": np.zeros(C, np.float32),
        "Wo": (rng.standard_normal((C, C)) / 16).astype(np.float32),
        "bo": np.zeros(C, np.float32),
    }
    o = kernel(**ins)
    print("kernel ran, out shape", o.shape, "absmax", np.abs(o).max())

